# revision 25
# baseline (speedup 1.0000x reference)
# Trainium2 Bass kernel for the BronxLayer GNN message-passing problem (v2.1).
#
# Reference math (fp32):
#   hn = LayerNorm(h)*gamma + beta ; xn = x / max(|x|_1, 1e-12)
#   k = hn@w_k.T ; q = hn@w_q.T ; a_h = softmax(k@q.T/16) ; a_x = xn@xn.T
#   i = [diag(a_x), rowsum(a_x), rowstd(a_x, ddof=1)] ; m = softmax(mixing, 0)
#   x_out = (m00*a_x + m10*a_h)@xn + x
#   h_agg = m01*(a_x@hn) + m11*(a_h.T@hn)          (a_x symmetric)
#   h_out = elu([h_agg|i]@w_v.T) + h
#
# Sharding: nodes row-sharded over 8 cores.  The only cross-core term,
# m11*(a_h.T@hn), is reduced with one bf16 ReduceScatter of
# partial = E.T @ [hn*m11/rowsum | m11/rowsum], triggered right after the
# h-projection path so it overlaps the whole x/aggregation phase.
#
# Perf structure (v2.1):
#  - fp8e4+DoubleRow matmuls for qT/kT/S/a_xc/ps_xs/Gram/GX; E kept bf16 so
#    ET comes from 4 hardware DMA transposes instead of 64 PE transposes.
#  - per-chunk pipelined LayerNorm: bn_stats/bn_aggr + a bit-trick Newton
#    rsqrt on the vector engine (no scalar Sqrt -> scalar act table stays
#    on Exp the whole kernel, zero table reloads after the first).
#  - engine balance: applies on gpsimd, psum->sbuf casts split
#    scalar/vector, input DMA issue split across the two HWDGE queues.
import sys

if "/opt/trn_rl_repo" not in sys.path:
    sys.path.insert(0, "/opt/trn_rl_repo")

import numpy as np

N, F = 4096, 256
NCORES = 8
R = N // NCORES  # 512
P = 128
MT = R // P      # 4
NT = N // P      # 32
FT = F // P      # 2
NCH = N // 512   # 8
FP = 272         # partial width: F cols + colsum col + pad (16B-aligned)
LN_EPS = 1e-5
L1_EPS = 1e-12
WS = 8.0                     # w_k / w_q prescale (host)
SEXP = (1.0 / 16.0) / (WS * WS)
SA = 256.0                   # xn fp8 scale
SA2 = 8192.0                 # a_x fp8 scale (psum SA^2*a_x -> copy scale 1/8)
SG = 1024.0                  # Gram fp8 scale
SS = 0.25                    # colsum fp8 scale relative to SA*colsum
RSQRT_MAGIC = 0x5F3759DF

_CACHE = {}


def _build():
    import contextlib

    import concourse.mybir as mybir
    import concourse.tile as tile
    from concourse import bacc
    from concourse.bass import ds, ts
    from concourse.masks import make_identity

    f32 = mybir.dt.float32
    f32r = mybir.dt.float32r
    bf16 = mybir.dt.bfloat16
    f8 = mybir.dt.float8e4
    i32 = mybir.dt.int32
    AF = mybir.ActivationFunctionType
    OP = mybir.AluOpType
    AX = mybir.AxisListType
    DR = mybir.MatmulPerfMode.DoubleRow

    nc = bacc.Bacc(None, num_devices=NCORES)

    h_ext = nc.declare_dram_parameter("h", [N, F], f32, isOutput=False)
    x_ext = nc.declare_dram_parameter("x", [N, F], f32, isOutput=False)
    hloc_ext = nc.declare_dram_parameter("h_loc", [R, F], f32, isOutput=False)
    xloc_ext = nc.declare_dram_parameter("x_loc", [R, F], f32, isOutput=False)
    wk_ext = nc.declare_dram_parameter("w_k8", [F, F], f32, isOutput=False)
    wq_ext = nc.declare_dram_parameter("w_q8", [F, F], f32, isOutput=False)
    wv_ext = nc.declare_dram_parameter("w_vTm", [F, F], f32r, isOutput=False)
    wvt_ext = nc.declare_dram_parameter("wv_tail", [4, F], f32r, isOutput=False)
    mn_ext = nc.declare_dram_parameter("m_n", [4], f32, isOutput=False)
    gam_ext = nc.declare_dram_parameter("ln_gamma", [F], f32, isOutput=False)
    bet_ext = nc.declare_dram_parameter("ln_beta", [F], f32, isOutput=False)
    hout_ext = nc.declare_dram_parameter("h_out", [R, F], f32, isOutput=True)
    xout_ext = nc.declare_dram_parameter("x_out", [R, F], f32, isOutput=True)

    h_v = h_ext.rearrange("(o p) f -> p o f", p=P)
    x_v = x_ext.rearrange("(o p) f -> p o f", p=P)
    hloc_v = hloc_ext.rearrange("(o p) f -> p o f", p=P)
    xloc_v = xloc_ext.rearrange("(o p) f -> p o f", p=P)
    hout_v = hout_ext.rearrange("(o p) f -> p o f", p=P)
    xout_v = xout_ext.rearrange("(o p) f -> p o f", p=P)

    NS = NT + MT

    with tile.TileContext(nc) as tc, contextlib.ExitStack() as ctx:
        const = ctx.enter_context(tc.tile_pool(name="const", bufs=1))
        persist = ctx.enter_context(tc.tile_pool(name="persist", bufs=1))
        dram = ctx.enter_context(tc.tile_pool(name="dram", bufs=1, space="DRAM"))
        small = ctx.enter_context(tc.tile_pool(name="small", bufs=3))

        # ---------------- persistent tensors ----------------
        h_in = persist.tile([P, NT, F], f32, name="h_in")
        x_in = persist.tile([P, NT, F], f32, name="x_in")
        h_lin = persist.tile([P, MT, F], f32, name="h_lin")
        x_lin = persist.tile([P, MT, F], f32, name="x_lin")
        xh_b = persist.tile([P, NT, 2 * F], f8, name="xh_b")
        xnT = persist.tile([P, FT, N], f8, name="xnT")
        kT_loc = persist.tile([P, FT, R], f8, name="kT_loc")
        xnT_loc = persist.tile([P, FT, R], f8, name="xnT_loc")
        E = persist.tile([P, MT, N], bf16, name="E")
        ET_all = persist.tile([P, NT, R], bf16, name="ET_all")
        hn_loc = persist.tile([P, MT, F], bf16, name="hn_loc")
        xn_loc_b = persist.tile([P, MT, F], bf16, name="xn_loc_b")
        hn_scaled = persist.tile([P, MT, FP], bf16, name="hn_scaled")
        rowsum_parts = persist.tile([P, MT, NCH], f32, name="rowsum_parts")
        recip_r = persist.tile([P, MT], f32, name="recip_r")
        diag = persist.tile([P, MT], f32, name="diag")
        srow = persist.tile([P, MT], f32, name="srow")
        stdv = persist.tile([P, MT], f32, name="stdv")
        G8 = persist.tile([P, FT, F], f8, name="G8")
        GXT = persist.tile([P, MT, F], bf16, name="GXT")
        i_cols = persist.tile([P, MT, 4], f32, name="i_cols")
        i_T = persist.tile([P, R], f32r, name="i_T")
        rs_sb = persist.tile([P, MT, FP], bf16, name="rs_sb")
        h_aggT = persist.tile([P, FT, R], f32r, name="h_aggT")
        l1_t = persist.tile([P, NS], f32, name="l1_t")
        rl1s_t = persist.tile([P, NS], f32, name="rl1s_t")

        partial_dram = dram.tile([N, FP], bf16, name="partial_dram")
        rs_dram = dram.tile([R, FP], bf16, name="rs_dram")
        pd_v = partial_dram.rearrange("(a p) f -> p a f", p=P)

        # ---------------- DMA issue: h on sync, x/weights on scalar --------
        for c in range(NCH):
            nc.sync.dma_start(h_in[:, ds(4 * c, 4), :], h_v[:, ds(4 * c, 4)])
        nc.sync.dma_start(h_lin[:], hloc_v[:])
        for c in range(NCH):
            nc.scalar.dma_start(x_in[:, ds(4 * c, 4), :], x_v[:, ds(4 * c, 4)])
        nc.scalar.dma_start(x_lin[:], xloc_v[:])

        m_bc = const.tile([P, 4], f32, name="m_bc")
        nc.sync.dma_start(m_bc[:], mn_ext.rearrange("a -> () a").to_broadcast((P, 4)))
        M00, M01, M10, M11 = (m_bc[:, j : j + 1] for j in range(4))
        gam_f = const.tile([P, FT, 1], f32, name="gam_f")
        nc.sync.dma_start(gam_f[:, :, 0], gam_ext.rearrange("(o p) -> p o", p=P))
        bet_f = const.tile([P, FT, 1], f32, name="bet_f")
        nc.sync.dma_start(bet_f[:, :, 0], bet_ext.rearrange("(o p) -> p o", p=P))
        wvT = const.tile([P, FT, F], f32r, name="wvT")
        nc.scalar.dma_start(wvT[:], wv_ext.rearrange("(o p) f -> p o f", p=P))
        wvT3 = const.tile([P, F], f32r, name="wvT3")
        nc.vector.memset(wvT3[:].bitcast(f32), 0.0)
        nc.scalar.dma_start(wvT3[:4, :], wvt_ext[:])

        ident8 = const.tile([P, P], f8, name="ident8")
        make_identity(nc, ident8)
        ident_b = const.tile([P, P], bf16, name="ident_b")
        make_identity(nc, ident_b)
        ident_f = const.tile([P, P], f32, name="ident_f")
        make_identity(nc, ident_f)

        wq8 = const.tile([P, FT, F], f8, name="wq8")
        wk8 = const.tile([P, FT, F], f8, name="wk8")

        rm01 = const.tile([P, 1], f32, name="rm01")
        nc.vector.reciprocal(rm01[:], M01)
        gam_eff = const.tile([P, FT, 1], f32, name="gam_eff")
        for ft in range(FT):
            nc.vector.tensor_tensor(gam_eff[:, ft], gam_f[:, ft], rm01[:], OP.mult)
        m00s = const.tile([P, 1], f32, name="m00s")
        nc.vector.tensor_scalar_mul(m00s[:], M00, 1.0 / (SA2 * SA))
        m10s = const.tile([P, 1], f32, name="m10s")
        nc.vector.tensor_scalar_mul(m10s[:], M10, 1.0 / SA)

        def newton_rsqrt(ve, nrows, tag):
            """1/sqrt(ve) on the vector engine (ve: [P, nrows] f32 AP,
            destroyed).  Returns an f32 AP view.  ~7 small DVE ops."""
            ish = small.tile([P, nrows], i32, name="nr_i" + tag, tag="nri" + tag)
            nc.vector.tensor_scalar(
                ish[:], ve.bitcast(i32), scalar1=1, scalar2=None,
                op0=OP.logical_shift_right,
            )
            y0 = small.tile([P, nrows], i32, name="nr_y" + tag, tag="nry" + tag)
            nc.vector.tensor_scalar(
                y0[:], ish[:], scalar1=-1, scalar2=RSQRT_MAGIC, op0=OP.mult,
                op1=OP.add,
            )
            yf = y0[:].bitcast(f32)
            hv = small.tile([P, nrows], f32, name="nr_h" + tag, tag="nrh" + tag)
            nc.vector.tensor_scalar_mul(hv[:], ve, 0.5)
            t = small.tile([P, nrows], f32, name="nr_t" + tag, tag="nrt" + tag)
            for _ in range(2):
                nc.vector.tensor_tensor(t[:], yf, yf, OP.mult)
                nc.vector.tensor_tensor(t[:], t[:], hv[:], OP.mult)
                nc.vector.tensor_scalar(
                    t[:], t[:], scalar1=-1.0, scalar2=1.5, op0=OP.mult, op1=OP.add
                )
                nc.vector.tensor_tensor(yf, yf, t[:], OP.mult)
            return yf

        def ln_coeffs(src_blk, nrows, tag):
            """bn_stats -> (rstm, nmrm) = (m01/sd, -mean*m01/sd) and
            (rstd, nmr) plain views for the local tile."""
            st6 = small.tile([P, nrows, 6], f32, name="st6" + tag, tag="st6")
            for j in range(nrows):
                nc.vector.bn_stats(st6[:, j], src_blk[:, j])
            mv = small.tile([P, nrows, 2], f32, name="mv" + tag, tag="mv")
            for j in range(nrows):
                nc.vector.bn_aggr(mv[:, j], st6[:, j])
            ve = small.tile([P, nrows], f32, name="ve" + tag, tag="ve")
            nc.vector.tensor_scalar_add(ve[:], mv[:, :, 1], LN_EPS)
            rstd = newton_rsqrt(ve[:], nrows, tag)
            rstm = small.tile([P, nrows], f32, name="rsm" + tag, tag="rsm")
            nc.vector.tensor_tensor(
                rstm[:], rstd, M01.to_broadcast((P, nrows)), OP.mult
            )
            nmrm = small.tile([P, nrows], f32, name="nmm" + tag, tag="nmm")
            nc.vector.scalar_tensor_tensor(
                nmrm[:], mv[:, :, 0], -1.0, rstm[:], OP.mult, OP.mult
            )
            return rstd, mv, rstm, nmrm

        # ============ h phase ============
        with tc.tile_pool(name="p1", bufs=1, space="PSUM") as p1, \
             tc.tile_pool(name="sc1", bufs=1) as sc1:
            # weight staging + fp8 conversion (gpsimd; vector stays free)
            wq_st = sc1.tile([P, FT, F], f32, name="wq_st", tag="wst", bufs=1)
            nc.sync.dma_start(wq_st[:], wq_ext.rearrange("(o p) f -> p o f", p=P))
            nc.gpsimd.tensor_copy(out=wq8[:], in_=wq_st[:])
            wk_st = sc1.tile([P, FT, F], f32, name="wk_st", tag="wst", bufs=1)
            nc.sync.dma_start(wk_st[:], wk_ext.rearrange("(o p) f -> p o f", p=P))
            nc.gpsimd.tensor_copy(out=wk8[:], in_=wk_st[:])
            # ---- local tile: hn_loc, lh8, hnT_l, kT_loc ----
            rstd_l, mv_l, rstm_l, nmrm_l = ln_coeffs(h_lin[:], MT, "l")
            lh8 = sc1.tile([P, MT, F], f8, name="lh8")
            for j in range(MT):
                nc.gpsimd.tensor_scalar(
                    lh8[:, j], h_lin[:, j],
                    scalar1=rstm_l[:, j : j + 1], scalar2=nmrm_l[:, j : j + 1],
                    op0=OP.mult, op1=OP.add,
                )
            nmr_l = small.tile([P, MT], f32, name="nmr_l", tag="nmrl", bufs=1)
            nc.vector.scalar_tensor_tensor(
                nmr_l[:], mv_l[:, :, 0], -1.0, rstd_l, OP.mult, OP.mult
            )
            for j in range(MT):
                nc.gpsimd.tensor_scalar(
                    hn_loc[:, j], h_lin[:, j],
                    scalar1=rstd_l[:, j : j + 1], scalar2=nmr_l[:, j : j + 1],
                    op0=OP.mult, op1=OP.add,
                )
            hnT_l = sc1.tile([P, FT, R], f8, name="hnT_l")
            for ft in range(FT):
                ps_t = p1.tile([P, R, 2], f8, name="ps_tl", tag="tp", bufs=2)
                for j in range(MT):
                    nc.tensor.transpose(
                        ps_t[:, ts(j, P), 0], lh8[:, j, ds(128 * ft, P)], ident8[:]
                    )
                nc.vector.tensor_scalar(
                    hnT_l[:, ft], ps_t[:, :, 0],
                    scalar1=gam_eff[:, ft], scalar2=bet_f[:, ft],
                    op0=OP.mult, op1=OP.add,
                )
            for fo in range(FT):
                ps_k = p1.tile([P, R], f32, name="ps_k", tag="mm", bufs=4)
                nc.tensor.matmul(
                    ps_k[:], wk8[:, 0:2, ds(128 * fo, P)], hnT_l[:, 0:2, :],
                    start=True, stop=True, perf_mode=DR,
                )
                nc.scalar.activation(kT_loc[:, fo], ps_k[:], AF.Copy)

            # ---- global chunks, fully pipelined ----
            for c in range(NCH):
                blk = h_in[:, ds(4 * c, 4), :]
                _, _, rstm, nmrm = ln_coeffs(blk, 4, "c")
                for j in range(4):
                    nt = 4 * c + j
                    nc.gpsimd.tensor_scalar(
                        xh_b[:, nt, F : 2 * F], h_in[:, nt],
                        scalar1=rstm[:, j : j + 1], scalar2=nmrm[:, j : j + 1],
                        op0=OP.mult, op1=OP.add,
                    )
                hnT_c = sc1.tile([P, FT, R], f8, name="hnT_c", tag="hnT", bufs=2)
                for ft in range(FT):
                    ps_t = p1.tile([P, R, 2], f8, name="ps_t", tag="tp", bufs=2)
                    for j in range(4):
                        nt = 4 * c + j
                        nc.tensor.transpose(
                            ps_t[:, ts(j, P), 0],
                            xh_b[:, nt, ds(F + 128 * ft, P)],
                            ident8[:],
                        )
                    nc.vector.tensor_scalar(
                        hnT_c[:, ft], ps_t[:, :, 0],
                        scalar1=gam_eff[:, ft], scalar2=bet_f[:, ft],
                        op0=OP.mult, op1=OP.add,
                    )
                qT_c = sc1.tile([P, FT, R], f8, name="qT_c", tag="qTc", bufs=2)
                for fo in range(FT):
                    ps_q = p1.tile([P, R], f32, name="ps_q", tag="mm", bufs=4)
                    nc.tensor.matmul(
                        ps_q[:], wq8[:, 0:2, ds(128 * fo, P)], hnT_c[:, 0:2, :],
                        start=True, stop=True, perf_mode=DR,
                    )
                    nc.scalar.activation(qT_c[:, fo], ps_q[:], AF.Copy)
                for mt in range(MT):
                    ps_s = p1.tile([P, R], f32, name="ps_s", tag="mm", bufs=4)
                    nc.tensor.matmul(
                        ps_s[:],
                        kT_loc[:, 0:2, ds(128 * mt, P)],
                        qT_c[:, 0:2, :],
                        start=True, stop=True, perf_mode=DR,
                    )
                    nc.scalar.activation(
                        E[:, mt, ds(512 * c, 512)], ps_s[:], AF.Exp,
                        scale=SEXP, accum_out=rowsum_parts[:, mt, c : c + 1],
                    )

            # ---- rowsums -> hn_scaled ----
            rs1 = small.tile([P, MT], f32, name="rs1", tag="rs1")
            nc.vector.tensor_reduce(rs1[:], rowsum_parts[:], AX.X, OP.add)
            nc.vector.reciprocal(recip_r[:], rs1[:])
            sch = small.tile([P, MT], f32, name="sch", tag="sch", bufs=1)
            nc.vector.tensor_tensor(
                sch[:], recip_r[:], M11.to_broadcast((P, MT)), OP.mult
            )
            nc.vector.memset(hn_scaled[:].bitcast(f32), 0.0)
            for mt in range(MT):
                nc.vector.tensor_scalar_mul(
                    hn_scaled[:, mt, 0:F], hn_loc[:, mt], sch[:, mt : mt + 1]
                )
                nc.vector.tensor_copy(
                    out=hn_scaled[:, mt, F : F + 1], in_=sch[:, mt : mt + 1]
                )

            # ---- partial = E.T @ hn_scaled -> bf16 -> DRAM (scalar stg) ----
            for g in range(8):
                stg = sc1.tile([P, 4, FP], bf16, name="stg", tag="stg", bufs=2)
                for k in range(4):
                    ic = 4 * g + k
                    ps_p = p1.tile([P, FP], f32, name="ps_p", tag="mm", bufs=4)
                    for jt in range(MT):
                        nc.tensor.matmul(
                            ps_p[:],
                            E[:, jt, ds(128 * ic, P)],
                            hn_scaled[:, jt, :],
                            start=(jt == 0), stop=(jt == MT - 1),
                        )
                    nc.scalar.activation(stg[:, k], ps_p[:], AF.Copy)
                nc.sync.dma_start(pd_v[:, ds(4 * g, 4), :], stg[:])

        nc.gpsimd.collective_compute(
            "ReduceScatter",
            mybir.AluOpType.add,
            replica_groups=[list(range(NCORES))],
            ins=[partial_dram[:]],
            outs=[rs_dram[:]],
        )
        nc.sync.dma_start(rs_sb[:], rs_dram.rearrange("(o p) f -> p o f", p=P))

        # ============ x phase ============
        with tc.tile_pool(name="p2", bufs=1, space="PSUM") as p2, \
             tc.tile_pool(name="sc2", bufs=1) as sc2:
            # L1 (vector has slack while partial streams out)
            for c in range(NCH):
                nc.vector.tensor_reduce(
                    l1_t[:, ds(4 * c, 4)], x_in[:, ds(4 * c, 4), :], AX.X, OP.add,
                    apply_absolute_value=True,
                )
            nc.vector.tensor_reduce(
                l1_t[:, ds(NT, MT)], x_lin[:], AX.X, OP.add,
                apply_absolute_value=True,
            )
            nc.vector.tensor_scalar_max(l1_t[:], l1_t[:], L1_EPS)
            nc.vector.reciprocal(rl1s_t[:], l1_t[:])
            nc.vector.tensor_scalar_mul(rl1s_t[:], rl1s_t[:], SA)

            # local rows: xn_loc_b (bf16), x8_l (fp8), diag, xnT_loc
            rl1p = small.tile([P, MT], f32, name="rl1p", tag="rl1p", bufs=1)
            nc.vector.tensor_scalar_mul(rl1p[:], rl1s_t[:, ds(NT, MT)], 1.0 / SA)
            sqj = sc2.tile([P, MT, F], f8, name="sqj")
            ssq = small.tile([P, MT], f32, name="ssq", tag="ssq", bufs=1)
            for j in range(MT):
                nc.gpsimd.tensor_scalar_mul(
                    xn_loc_b[:, j], x_lin[:, j], rl1p[:, j : j + 1]
                )
                nc.scalar.activation(
                    sqj[:, j], x_lin[:, j], AF.Square,
                    accum_out=ssq[:, j : j + 1],
                )
            t0 = small.tile([P, MT], f32, name="t0d", tag="t0d", bufs=1)
            nc.vector.tensor_tensor(t0[:], rl1p[:], rl1p[:], OP.mult)
            nc.vector.tensor_tensor(diag[:], ssq[:], t0[:], OP.mult)
            x8_l = sc2.tile([P, MT, F], f8, name="x8_l")
            for j in range(MT):
                nc.gpsimd.tensor_scalar_mul(
                    x8_l[:, j], x_lin[:, j], rl1s_t[:, NT + j : NT + j + 1]
                )
            for ft in range(FT):
                ps_t = p2.tile([P, R, 2], f8, name="ps_xtl", tag="tp", bufs=2)
                for j in range(MT):
                    nc.tensor.transpose(
                        ps_t[:, ts(j, P), 0], x8_l[:, j, ds(128 * ft, P)], ident8[:]
                    )
                nc.vector.tensor_copy(out=xnT_loc[:, ft], in_=ps_t[:, :, 0])

            # global: xh_b x-half (gpsimd), xnT (vector casts), Gram (DR)
            ps_G = p2.tile([P, 2 * F], f32, name="ps_G", tag="G", bufs=1)
            for c in range(NCH):
                for j in range(4):
                    nt = 4 * c + j
                    nc.gpsimd.tensor_scalar_mul(
                        xh_b[:, nt, 0:F], x_in[:, nt], rl1s_t[:, nt : nt + 1]
                    )
                for ft in range(FT):
                    ps_t = p2.tile([P, R, 2], f8, name="ps_xt", tag="tp", bufs=2)
                    for j in range(4):
                        nt = 4 * c + j
                        nc.tensor.transpose(
                            ps_t[:, ts(j, P), 0],
                            xh_b[:, nt, ds(128 * ft, P)],
                            ident8[:],
                        )
                    nc.vector.tensor_copy(
                        out=xnT[:, ft, ds(512 * c, 512)], in_=ps_t[:, :, 0]
                    )
                for pr in range(2):
                    nt0 = 4 * c + 2 * pr
                    for m in range(FT):
                        nc.tensor.matmul(
                            ps_G[:, ts(m, F)],
                            xh_b[:, nt0 : nt0 + 2, ds(128 * m, P)],
                            xh_b[:, nt0 : nt0 + 2, 0:F],
                            start=(c == 0 and pr == 0),
                            stop=(c == NCH - 1 and pr == 1),
                            perf_mode=DR,
                        )
            for m in range(FT):
                nc.vector.tensor_scalar_mul(
                    G8[:, m], ps_G[:, ts(m, F)], SG / (SA * SA)
                )

            # ET via 4 hardware DMA transposes (bf16): E[:, mt, :] -> columns
            for mt in range(MT):
                nc.sync.dma_start_transpose(
                    ET_all[:, :, ds(128 * mt, P)], E[:, mt, :]
                )

        # ============ aggregation ============
        with tc.tile_pool(name="p3", bufs=1, space="PSUM") as p3, \
             tc.tile_pool(name="sc3", bufs=1) as sc3:
            ps_xs = [
                p3.tile([P, R], f32, name=f"ps_xs{mt}", tag=f"xs{mt}", bufs=1)
                for mt in range(MT)
            ]
            with tc.tile_pool(name="p3a", bufs=1, space="PSUM") as p3a:
                for icp in range(16):
                    axc2 = sc3.tile([P, 2, R], f8, name="axc2", tag="axc", bufs=3)
                    for q in range(2):
                        ic = 2 * icp + q
                        ps_ax = p3a.tile([P, R], f32, name="ps_ax", tag="ax",
                                         bufs=4)
                        nc.tensor.matmul(
                            ps_ax[:], xnT[:, 0:2, ds(128 * ic, P)],
                            xnT_loc[:, 0:2, :],
                            start=True, stop=True, perf_mode=DR,
                        )
                        eng = nc.scalar if q == 0 else nc.vector
                        if q == 0:
                            nc.scalar.activation(
                                axc2[:, q], ps_ax[:], AF.Copy,
                                scale=SA2 / (SA * SA),
                            )
                        else:
                            nc.vector.tensor_scalar_mul(
                                axc2[:, q], ps_ax[:], SA2 / (SA * SA)
                            )
                    for mt in range(MT):
                        nc.tensor.matmul(
                            ps_xs[mt][:],
                            axc2[:, 0:2, ds(128 * mt, P)],
                            xh_b[:, ds(2 * icp, 2), :],
                            start=(icp == 0), stop=(icp == 15), perf_mode=DR,
                        )

            # bslcT: (E @ xn).T accumulated f-major (fp8 lhsT x bf16 rhs)
            p3b_cm = tc.tile_pool(name="p3b", bufs=1, space="PSUM")
            p3b = p3b_cm.__enter__()
            ps_bT = [
                p3b.tile([P, R], f32, name=f"ps_bT{fs}", tag=f"bT{fs}", bufs=1)
                for fs in range(FT)
            ]
            for ic in range(NT):
                for fs in range(FT):
                    nc.tensor.matmul(
                        ps_bT[fs][:],
                        xh_b[:, ic, ds(128 * fs, P)],
                        ET_all[:, ic, :],
                        start=(ic == 0), stop=(ic == NT - 1),
                    )
            bT_sb = sc3.tile([P, FT, R], bf16, name="bT_sb")
            for fs in range(FT):
                nc.vector.tensor_copy(out=bT_sb[:, fs], in_=ps_bT[fs][:])

            # ---- x_out combine ----
            for mt in range(MT):
                ps_bb = p3b.tile([P, F], bf16, name="ps_bb", tag="w", bufs=2)
                for fs in range(FT):
                    nc.tensor.transpose(
                        ps_bb[:, ts(fs, P)], bT_sb[:, fs, ds(128 * mt, P)],
                        ident_b[:],
                    )
                xo = small.tile([P, F], f32, name="xo", tag="xo", bufs=2)
                nc.vector.tensor_scalar_mul(xo[:], ps_xs[mt][:, 0:F], m00s[:])
                scb = small.tile([P, 1], f32, name="scb", tag="scb")
                nc.vector.tensor_tensor(
                    scb[:], recip_r[:, mt : mt + 1], m10s[:], OP.mult
                )
                tb = small.tile([P, F], f32, name="tb", tag="wkb", bufs=2)
                nc.vector.tensor_scalar_mul(tb[:], ps_bb[:], scb[:])
                nc.vector.tensor_tensor(xo[:], xo[:], tb[:], OP.add)
                nc.vector.tensor_tensor(xo[:], xo[:], x_lin[:, mt], OP.add)
                nc.sync.dma_start(xout_v[:, mt], xo[:])

            # ---- late stats: srow / GX -> sumsq -> std ----
            s_f = small.tile([P, FT], f32, name="s_f", tag="s_f", bufs=1)
            nc.vector.tensor_reduce(s_f[:], xnT[:], AX.X, OP.add)
            s8 = small.tile([P, FT], f8, name="s8", tag="s8", bufs=1)
            nc.vector.tensor_scalar_mul(s8[:], s_f[:], SS)
            ps_sr = p3b.tile([P, MT], f32, name="ps_sr", tag="w", bufs=2)
            for mt in range(MT):
                for ft in range(FT):
                    nc.tensor.matmul(
                        ps_sr[:, mt : mt + 1],
                        xnT_loc[:, ft, ds(128 * mt, P)],
                        s8[:, ft : ft + 1],
                        start=(ft == 0), stop=(ft == 1),
                    )
            nc.vector.tensor_scalar_mul(srow[:], ps_sr[:], 1.0 / (SA * SA * SS))
            GX_sb = sc3.tile([P, FT, R], bf16, name="GX_sb")
            for fs in range(FT):
                ps_gx = p3b.tile([P, R], f32, name="ps_gx", tag="w", bufs=2)
                nc.tensor.matmul(
                    ps_gx[:], G8[:, 0:2, ds(128 * fs, P)], xnT_loc[:, 0:2, :],
                    start=True, stop=True, perf_mode=DR,
                )
                nc.vector.tensor_copy(out=GX_sb[:, fs], in_=ps_gx[:])
            for mt in range(MT):
                ps_gxt = p3b.tile([P, F], bf16, name="ps_gxt", tag="w", bufs=2)
                for fs in range(FT):
                    nc.tensor.transpose(
                        ps_gxt[:, ts(fs, P)], GX_sb[:, fs, ds(128 * mt, P)],
                        ident_b[:],
                    )
                nc.vector.tensor_copy(out=GXT[:, mt], in_=ps_gxt[:])
            sumsq = small.tile([P, MT], f32, name="sumsq", tag="sumsq", bufs=1)
            for mt in range(MT):
                tmp2 = small.tile([P, F], f32, name="tmp2", tag="wka", bufs=2)
                nc.vector.tensor_tensor(
                    tmp2[:], xn_loc_b[:, mt], GXT[:, mt], OP.mult
                )
                nc.vector.tensor_reduce(
                    sumsq[:, mt : mt + 1], tmp2[:], AX.X, OP.add
                )
            sr2 = small.tile([P, MT], f32, name="sr2", tag="sr2", bufs=1)
            nc.vector.scalar_tensor_tensor(
                sr2[:], srow[:], 1.0 / (N * (N - 1.0)), srow[:], OP.mult, OP.mult
            )
            t1 = small.tile([P, MT], f32, name="t1", tag="t1", bufs=1)
            nc.vector.scalar_tensor_tensor(
                t1[:], sumsq[:], 1.0 / (SG * SA * (N - 1.0)), sr2[:],
                OP.mult, OP.subtract,
            )
            # std = t1 * rsqrt(t1) (vector; avoids a scalar Sqrt table load)
            t1c = small.tile([P, MT], f32, name="t1c", tag="t1c", bufs=1)
            nc.vector.tensor_scalar_max(t1c[:], t1[:], 1e-30)
            rsq = newton_rsqrt(t1c[:], MT, "s")
            nc.vector.tensor_tensor(stdv[:], t1[:], rsq, OP.mult)

            # ---- h path ----
            h_agg_s = sc3.tile([P, MT, F], bf16, name="h_agg_s")
            for mt in range(MT):
                nc.vector.scalar_tensor_tensor(
                    h_agg_s[:, mt], ps_xs[mt][:, F : 2 * F], 1.0 / SA2,
                    rs_sb[:, mt, 0:F], OP.mult, OP.add,
                )
                nc.vector.tensor_copy(out=i_cols[:, mt, 0:1], in_=diag[:, mt : mt + 1])
                nc.vector.tensor_copy(out=i_cols[:, mt, 1:2], in_=srow[:, mt : mt + 1])
                nc.vector.tensor_copy(out=i_cols[:, mt, 2:3], in_=stdv[:, mt : mt + 1])
                c4 = small.tile([P, 1], f32, name="c4", tag="c4", bufs=4)
                nc.vector.tensor_tensor(c4[:], srow[:, mt : mt + 1], M01, OP.mult)
                nc.vector.tensor_tensor(
                    i_cols[:, mt, 3:4], rs_sb[:, mt, F : F + 1], c4[:], OP.add
                )
            nc.vector.memset(i_T[:].bitcast(f32), 0.0)
            for mt in range(MT):
                ps_i = p3b.tile([4, P], f32, name="ps_i", tag="w", bufs=2)
                nc.tensor.transpose(ps_i[:], i_cols[:, mt], ident_f[:])
                nc.vector.tensor_copy(out=i_T[:4, ds(128 * mt, P)], in_=ps_i[:])
            for ft in range(FT):
                ps_hat = p3b.tile([P, R], bf16, name="ps_hat", tag="w", bufs=2)
                for mt in range(MT):
                    nc.tensor.transpose(
                        ps_hat[:, ts(mt, P)], h_agg_s[:, mt, ds(128 * ft, P)],
                        ident_b[:],
                    )
                nc.scalar.activation(
                    h_aggT[:, ft], ps_hat[:], AF.Copy, scale=gam_f[:, ft]
                )
            for mt in range(MT):
                ps_h = p3b.tile([P, F], f32, name="ps_h", tag="w", bufs=2)
                for k in range(FT):
                    nc.tensor.matmul(
                        ps_h[:], h_aggT[:, k, ds(128 * mt, P)], wvT[:, k],
                        start=(k == 0), stop=False,
                    )
                nc.tensor.matmul(
                    ps_h[:], i_T[:, ds(128 * mt, P)], wvT3[:],
                    start=False, stop=True,
                )
                vmin = small.tile([P, F], f32, name="vmin", tag="wka", bufs=2)
                nc.vector.tensor_scalar_min(vmin[:], ps_h[:], 0.0)
                ev = small.tile([P, F], f32, name="ev", tag="wkb", bufs=2)
                nc.scalar.activation(ev[:], vmin[:], AF.Exp)
                vmax = small.tile([P, F], f32, name="vmax", tag="wka", bufs=2)
                nc.vector.tensor_scalar_max(vmax[:], ps_h[:], 0.0)
                ho = small.tile([P, F], f32, name="ho", tag="ho", bufs=2)
                nc.vector.tensor_tensor(ho[:], ev[:], vmax[:], OP.add)
                nc.vector.tensor_scalar_add(ho[:], ho[:], -1.0)
                nc.vector.tensor_tensor(ho[:], ho[:], h_lin[:, mt], OP.add)
                nc.sync.dma_start(hout_v[:, mt], ho[:])
            p3b_cm.__exit__(None, None, None)

    nc.finalize()
    return nc


def _make_in_maps(inputs):
    h = np.ascontiguousarray(inputs["h"], dtype=np.float32)
    x = np.ascontiguousarray(inputs["x"], dtype=np.float32)
    w_k = np.asarray(inputs["w_k"], np.float32)
    w_q = np.asarray(inputs["w_q"], np.float32)
    w_v = np.asarray(inputs["w_v"], np.float32)
    mixing = np.asarray(inputs["mixing"], np.float32)
    gam = np.ascontiguousarray(inputs["ln_gamma"], dtype=np.float32)
    bet = np.asarray(inputs["ln_beta"], np.float32)

    w_k8 = np.ascontiguousarray(w_k.T * WS)
    w_q8 = np.ascontiguousarray(w_q.T * WS)
    w_vT = np.ascontiguousarray(w_v.T)  # [F+3, F]
    w_vTm = np.ascontiguousarray(w_vT[:F])
    bvec = w_v[:, :F] @ bet  # [F]
    wv_tail = np.ascontiguousarray(
        np.concatenate([w_vT[F:], bvec[None, :]], axis=0)
    )
    me = np.exp(mixing)
    m = me / me.sum(axis=0, keepdims=True)
    m_n = np.ascontiguousarray(m.reshape(-1))  # [m00, m01, m10, m11]

    return [
        {
            "h": h,
            "x": x,
            "h_loc": np.ascontiguousarray(h[c * R : (c + 1) * R]),
            "x_loc": np.ascontiguousarray(x[c * R : (c + 1) * R]),
            "w_k8": w_k8,
            "w_q8": w_q8,
            "w_vTm": w_vTm,
            "wv_tail": wv_tail,
            "m_n": m_n,
            "ln_gamma": gam,
            "ln_beta": np.ascontiguousarray(bet),
        }
        for c in range(NCORES)
    ]


def kernel(h, x, w_k, w_q, w_v, mixing, ln_gamma, ln_beta):
    from concourse.bass_utils import run_bass_kernel_spmd

    if "nc" not in _CACHE:
        _CACHE["nc"] = _build()
    nc = _CACHE["nc"]

    in_maps = _make_in_maps(
        {
            "h": h,
            "x": x,
            "w_k": w_k,
            "w_q": w_q,
            "w_v": w_v,
            "mixing": mixing,
            "ln_gamma": ln_gamma,
            "ln_beta": ln_beta,
        }
    )
    res = run_bass_kernel_spmd(nc, in_maps, list(range(NCORES))).results
    h_out = np.concatenate([res[c]["h_out"] for c in range(NCORES)], axis=0)
    x_out = np.concatenate([res[c]["x_out"] for c in range(NCORES)], axis=0)
    return (h_out, x_out)


# revision 26
# speedup vs baseline: 1.3629x; 1.3629x over previous
# Trainium2 Bass kernel for the BronxLayer GNN message-passing problem (v2.1).
#
# Reference math (fp32):
#   hn = LayerNorm(h)*gamma + beta ; xn = x / max(|x|_1, 1e-12)
#   k = hn@w_k.T ; q = hn@w_q.T ; a_h = softmax(k@q.T/16) ; a_x = xn@xn.T
#   i = [diag(a_x), rowsum(a_x), rowstd(a_x, ddof=1)] ; m = softmax(mixing, 0)
#   x_out = (m00*a_x + m10*a_h)@xn + x
#   h_agg = m01*(a_x@hn) + m11*(a_h.T@hn)          (a_x symmetric)
#   h_out = elu([h_agg|i]@w_v.T) + h
#
# Sharding: nodes row-sharded over 8 cores.  The only cross-core term,
# m11*(a_h.T@hn), is reduced with one bf16 ReduceScatter of
# partial = E.T @ [hn*m11/rowsum | m11/rowsum], triggered right after the
# h-projection path so it overlaps the whole x/aggregation phase.
#
# Perf structure (v2.1):
#  - fp8e4+DoubleRow matmuls for qT/kT/S/a_xc/ps_xs/Gram/GX; E kept bf16 so
#    ET comes from 4 hardware DMA transposes instead of 64 PE transposes.
#  - per-chunk pipelined LayerNorm: bn_stats/bn_aggr + a bit-trick Newton
#    rsqrt on the vector engine (no scalar Sqrt -> scalar act table stays
#    on Exp the whole kernel, zero table reloads after the first).
#  - engine balance: applies on gpsimd, psum->sbuf casts split
#    scalar/vector, input DMA issue split across the two HWDGE queues.
import sys

if "/opt/trn_rl_repo" not in sys.path:
    sys.path.insert(0, "/opt/trn_rl_repo")

import numpy as np

N, F = 4096, 256
NCORES = 8
R = N // NCORES  # 512
P = 128
MT = R // P      # 4
NT = N // P      # 32
FT = F // P      # 2
NCH = N // 512   # 8
FP = 272         # partial width: F cols + colsum col + pad (16B-aligned)
LN_EPS = 1e-5
L1_EPS = 1e-12
WS = 8.0                     # w_k / w_q prescale (host)
SEXP = (1.0 / 16.0) / (WS * WS)
SA = 256.0                   # xn fp8 scale
SA2 = 8192.0                 # a_x fp8 scale (psum SA^2*a_x -> copy scale 1/8)
SG = 1024.0                  # Gram fp8 scale
SS = 0.25                    # colsum fp8 scale relative to SA*colsum
RSQRT_MAGIC = 0x5F3759DF

_CACHE = {}


def _build():
    import contextlib

    import concourse.mybir as mybir
    import concourse.tile as tile
    from concourse import bacc
    from concourse.bass import ds, ts
    from concourse.masks import make_identity

    f32 = mybir.dt.float32
    f32r = mybir.dt.float32r
    bf16 = mybir.dt.bfloat16
    f8 = mybir.dt.float8e4
    i32 = mybir.dt.int32
    AF = mybir.ActivationFunctionType
    OP = mybir.AluOpType
    AX = mybir.AxisListType
    DR = mybir.MatmulPerfMode.DoubleRow

    nc = bacc.Bacc(None, num_devices=NCORES)

    h_ext = nc.declare_dram_parameter("h", [N, F], f32, isOutput=False)
    x_ext = nc.declare_dram_parameter("x", [N, F], f32, isOutput=False)
    hloc_ext = nc.declare_dram_parameter("h_loc", [R, F], f32, isOutput=False)
    xloc_ext = nc.declare_dram_parameter("x_loc", [R, F], f32, isOutput=False)
    wk_ext = nc.declare_dram_parameter("w_k8", [F, F], f32, isOutput=False)
    wq_ext = nc.declare_dram_parameter("w_q8", [F, F], f32, isOutput=False)
    wv_ext = nc.declare_dram_parameter("w_vTm", [F, F], f32r, isOutput=False)
    wvt_ext = nc.declare_dram_parameter("wv_tail", [4, F], f32r, isOutput=False)
    mn_ext = nc.declare_dram_parameter("m_n", [4], f32, isOutput=False)
    gam_ext = nc.declare_dram_parameter("ln_gamma", [F], f32, isOutput=False)
    bet_ext = nc.declare_dram_parameter("ln_beta", [F], f32, isOutput=False)
    hout_ext = nc.declare_dram_parameter("h_out", [R, F], f32, isOutput=True)
    xout_ext = nc.declare_dram_parameter("x_out", [R, F], f32, isOutput=True)

    h_v = h_ext.rearrange("(o p) f -> p o f", p=P)
    x_v = x_ext.rearrange("(o p) f -> p o f", p=P)
    hloc_v = hloc_ext.rearrange("(o p) f -> p o f", p=P)
    xloc_v = xloc_ext.rearrange("(o p) f -> p o f", p=P)
    hout_v = hout_ext.rearrange("(o p) f -> p o f", p=P)
    xout_v = xout_ext.rearrange("(o p) f -> p o f", p=P)

    NS = NT + MT

    with tile.TileContext(nc) as tc, contextlib.ExitStack() as ctx:
        const = ctx.enter_context(tc.tile_pool(name="const", bufs=1))
        persist = ctx.enter_context(tc.tile_pool(name="persist", bufs=1))
        dram = ctx.enter_context(tc.tile_pool(name="dram", bufs=1, space="DRAM"))
        small = ctx.enter_context(tc.tile_pool(name="small", bufs=3))

        # ---------------- persistent tensors ----------------
        h_in = persist.tile([P, NT, F], f32, name="h_in")
        x_in = persist.tile([P, NT, F], f32, name="x_in")
        h_lin = persist.tile([P, MT, F], f32, name="h_lin")
        x_lin = persist.tile([P, MT, F], f32, name="x_lin")
        xh_b = persist.tile([P, NT, 2 * F], f8, name="xh_b")
        xnT = persist.tile([P, FT, N], f8, name="xnT")
        kT_loc = persist.tile([P, FT, R], f8, name="kT_loc")
        xnT_loc = persist.tile([P, FT, R], f8, name="xnT_loc")
        E = persist.tile([P, MT, N], bf16, name="E")
        ET_all = persist.tile([P, NT, R], bf16, name="ET_all")
        hn_loc = persist.tile([P, MT, F], bf16, name="hn_loc")
        xn_loc_b = persist.tile([P, MT, F], bf16, name="xn_loc_b")
        hn_scaled = persist.tile([P, MT, FP], bf16, name="hn_scaled")
        rowsum_parts = persist.tile([P, MT, NCH], f32, name="rowsum_parts")
        recip_r = persist.tile([P, MT], f32, name="recip_r")
        diag = persist.tile([P, MT], f32, name="diag")
        srow = persist.tile([P, MT], f32, name="srow")
        stdv = persist.tile([P, MT], f32, name="stdv")
        G8 = persist.tile([P, FT, F], f8, name="G8")
        GXT = persist.tile([P, MT, F], bf16, name="GXT")
        i_cols = persist.tile([P, MT, 4], f32, name="i_cols")
        i_T = persist.tile([P, R], f32r, name="i_T")
        rs_sb = persist.tile([P, MT, FP], bf16, name="rs_sb")
        h_aggT = persist.tile([P, FT, R], f32r, name="h_aggT")
        l1_t = persist.tile([P, NS], f32, name="l1_t")
        rl1s_t = persist.tile([P, NS], f32, name="rl1s_t")

        partial_dram = dram.tile([N, FP], bf16, name="partial_dram")
        rs_dram = dram.tile([R, FP], bf16, name="rs_dram")
        pd_v = partial_dram.rearrange("(a p) f -> p a f", p=P)

        # ---------------- DMA issue: h on sync, x/weights on scalar --------
        for c in range(NCH):
            nc.sync.dma_start(h_in[:, ds(4 * c, 4), :], h_v[:, ds(4 * c, 4)])
        nc.sync.dma_start(h_lin[:], hloc_v[:])
        for c in range(NCH):
            nc.scalar.dma_start(x_in[:, ds(4 * c, 4), :], x_v[:, ds(4 * c, 4)])
        nc.scalar.dma_start(x_lin[:], xloc_v[:])

        m_bc = const.tile([P, 4], f32, name="m_bc")
        nc.sync.dma_start(m_bc[:], mn_ext.rearrange("a -> () a").to_broadcast((P, 4)))
        M00, M01, M10, M11 = (m_bc[:, j : j + 1] for j in range(4))
        gam_f = const.tile([P, FT, 1], f32, name="gam_f")
        nc.sync.dma_start(gam_f[:, :, 0], gam_ext.rearrange("(o p) -> p o", p=P))
        bet_f = const.tile([P, FT, 1], f32, name="bet_f")
        nc.sync.dma_start(bet_f[:, :, 0], bet_ext.rearrange("(o p) -> p o", p=P))
        wvT = const.tile([P, FT, F], f32r, name="wvT")
        nc.scalar.dma_start(wvT[:], wv_ext.rearrange("(o p) f -> p o f", p=P))
        wvT3 = const.tile([P, F], f32r, name="wvT3")
        nc.vector.memset(wvT3[:].bitcast(f32), 0.0)
        nc.scalar.dma_start(wvT3[:4, :], wvt_ext[:])

        ident8 = const.tile([P, P], f8, name="ident8")
        make_identity(nc, ident8)
        ident_b = const.tile([P, P], bf16, name="ident_b")
        make_identity(nc, ident_b)
        ident_f = const.tile([P, P], f32, name="ident_f")
        make_identity(nc, ident_f)

        wq8 = const.tile([P, FT, F], f8, name="wq8")
        wk8 = const.tile([P, FT, F], f8, name="wk8")

        rm01 = const.tile([P, 1], f32, name="rm01")
        nc.vector.reciprocal(rm01[:], M01)
        gam_eff = const.tile([P, FT, 1], f32, name="gam_eff")
        for ft in range(FT):
            nc.vector.tensor_tensor(gam_eff[:, ft], gam_f[:, ft], rm01[:], OP.mult)
        m00s = const.tile([P, 1], f32, name="m00s")
        nc.vector.tensor_scalar_mul(m00s[:], M00, 1.0 / (SA2 * SA))
        m10s = const.tile([P, 1], f32, name="m10s")
        nc.vector.tensor_scalar_mul(m10s[:], M10, 1.0 / SA)

        def newton_rsqrt(ve, nrows, tag):
            """1/sqrt(ve) on the vector engine (ve: [P, nrows] f32 AP,
            destroyed).  Returns an f32 AP view.  ~7 small DVE ops."""
            ish = small.tile([P, nrows], i32, name="nr_i" + tag, tag="nri" + tag)
            nc.vector.tensor_scalar(
                ish[:], ve.bitcast(i32), scalar1=1, scalar2=None,
                op0=OP.logical_shift_right,
            )
            y0 = small.tile([P, nrows], i32, name="nr_y" + tag, tag="nry" + tag)
            nc.vector.tensor_scalar(
                y0[:], ish[:], scalar1=-1, scalar2=RSQRT_MAGIC, op0=OP.mult,
                op1=OP.add,
            )
            yf = y0[:].bitcast(f32)
            hv = small.tile([P, nrows], f32, name="nr_h" + tag, tag="nrh" + tag)
            nc.vector.tensor_scalar_mul(hv[:], ve, 0.5)
            t = small.tile([P, nrows], f32, name="nr_t" + tag, tag="nrt" + tag)
            for _ in range(2):
                nc.vector.tensor_tensor(t[:], yf, yf, OP.mult)
                nc.vector.tensor_tensor(t[:], t[:], hv[:], OP.mult)
                nc.vector.tensor_scalar(
                    t[:], t[:], scalar1=-1.0, scalar2=1.5, op0=OP.mult, op1=OP.add
                )
                nc.vector.tensor_tensor(yf, yf, t[:], OP.mult)
            return yf

        def ln_coeffs(src_blk, nrows, tag):
            """bn_stats -> (rstm, nmrm) = (m01/sd, -mean*m01/sd) and
            (rstd, nmr) plain views for the local tile."""
            st6 = small.tile([P, nrows, 6], f32, name="st6" + tag, tag="st6")
            for j in range(nrows):
                nc.vector.bn_stats(st6[:, j], src_blk[:, j])
            mv = small.tile([P, nrows, 2], f32, name="mv" + tag, tag="mv")
            for j in range(nrows):
                nc.vector.bn_aggr(mv[:, j], st6[:, j])
            ve = small.tile([P, nrows], f32, name="ve" + tag, tag="ve")
            nc.vector.tensor_scalar_add(ve[:], mv[:, :, 1], LN_EPS)
            rstd = newton_rsqrt(ve[:], nrows, tag)
            rstm = small.tile([P, nrows], f32, name="rsm" + tag, tag="rsm")
            nc.vector.tensor_tensor(
                rstm[:], rstd, M01.to_broadcast((P, nrows)), OP.mult
            )
            nmrm = small.tile([P, nrows], f32, name="nmm" + tag, tag="nmm")
            nc.vector.scalar_tensor_tensor(
                nmrm[:], mv[:, :, 0], -1.0, rstm[:], OP.mult, OP.mult
            )
            return rstd, mv, rstm, nmrm

        # ============ h phase ============
        with tc.tile_pool(name="p1", bufs=1, space="PSUM") as p1, \
             tc.tile_pool(name="sc1", bufs=1) as sc1:
            # weight staging + fp8 conversion (gpsimd; vector stays free)
            wq_st = sc1.tile([P, FT, F], f32, name="wq_st", tag="wst", bufs=1)
            nc.sync.dma_start(wq_st[:], wq_ext.rearrange("(o p) f -> p o f", p=P))
            nc.vector.tensor_copy(out=wq8[:], in_=wq_st[:])
            wk_st = sc1.tile([P, FT, F], f32, name="wk_st", tag="wst", bufs=1)
            nc.sync.dma_start(wk_st[:], wk_ext.rearrange("(o p) f -> p o f", p=P))
            nc.vector.tensor_copy(out=wk8[:], in_=wk_st[:])
            # ---- local tile: hn_loc, lh8, hnT_l, kT_loc ----
            rstd_l, mv_l, rstm_l, nmrm_l = ln_coeffs(h_lin[:], MT, "l")
            lh8 = sc1.tile([P, MT, F], f8, name="lh8")
            for j in range(MT):
                nc.vector.tensor_scalar(
                    lh8[:, j], h_lin[:, j],
                    scalar1=rstm_l[:, j : j + 1], scalar2=nmrm_l[:, j : j + 1],
                    op0=OP.mult, op1=OP.add,
                )
            nmr_l = small.tile([P, MT], f32, name="nmr_l", tag="nmrl", bufs=1)
            nc.vector.scalar_tensor_tensor(
                nmr_l[:], mv_l[:, :, 0], -1.0, rstd_l, OP.mult, OP.mult
            )
            for j in range(MT):
                nc.vector.tensor_scalar(
                    hn_loc[:, j], h_lin[:, j],
                    scalar1=rstd_l[:, j : j + 1], scalar2=nmr_l[:, j : j + 1],
                    op0=OP.mult, op1=OP.add,
                )
            hnT_l = sc1.tile([P, FT, R], f8, name="hnT_l")
            for ft in range(FT):
                ps_t = p1.tile([P, R, 2], f8, name="ps_tl", tag="tp", bufs=2)
                for j in range(MT):
                    nc.tensor.transpose(
                        ps_t[:, ts(j, P), 0], lh8[:, j, ds(128 * ft, P)], ident8[:]
                    )
                nc.scalar.activation(
                    hnT_l[:, ft], ps_t[:, :, 0], AF.Identity,
                    bias=bet_f[:, ft], scale=gam_eff[:, ft],
                )
            for fo in range(FT):
                ps_k = p1.tile([P, R], f32, name="ps_k", tag="mm", bufs=4)
                nc.tensor.matmul(
                    ps_k[:], wk8[:, 0:2, ds(128 * fo, P)], hnT_l[:, 0:2, :],
                    start=True, stop=True, perf_mode=DR,
                )
                nc.scalar.activation(kT_loc[:, fo], ps_k[:], AF.Copy)

            # ---- global chunks, fully pipelined ----
            for c in range(NCH):
                blk = h_in[:, ds(4 * c, 4), :]
                _, _, rstm, nmrm = ln_coeffs(blk, 4, "c")
                for j in range(4):
                    nt = 4 * c + j
                    nc.vector.tensor_scalar(
                        xh_b[:, nt, F : 2 * F], h_in[:, nt],
                        scalar1=rstm[:, j : j + 1], scalar2=nmrm[:, j : j + 1],
                        op0=OP.mult, op1=OP.add,
                    )
                hnT_c = sc1.tile([P, FT, R], f8, name="hnT_c", tag="hnT", bufs=2)
                for ft in range(FT):
                    ps_t = p1.tile([P, R, 2], f8, name="ps_t", tag="tp", bufs=2)
                    for j in range(4):
                        nt = 4 * c + j
                        nc.tensor.transpose(
                            ps_t[:, ts(j, P), 0],
                            xh_b[:, nt, ds(F + 128 * ft, P)],
                            ident8[:],
                        )
                    nc.vector.tensor_scalar(
                        hnT_c[:, ft], ps_t[:, :, 0],
                        scalar1=gam_eff[:, ft], scalar2=bet_f[:, ft],
                        op0=OP.mult, op1=OP.add,
                    )
                qT_c = sc1.tile([P, FT, R], f8, name="qT_c", tag="qTc", bufs=2)
                for fo in range(FT):
                    ps_q = p1.tile([P, R], f32, name="ps_q", tag="mm", bufs=4)
                    nc.tensor.matmul(
                        ps_q[:], wq8[:, 0:2, ds(128 * fo, P)], hnT_c[:, 0:2, :],
                        start=True, stop=True, perf_mode=DR,
                    )
                    nc.scalar.activation(qT_c[:, fo], ps_q[:], AF.Copy)
                for mt in range(MT):
                    ps_s = p1.tile([P, R], f32, name="ps_s", tag="mm", bufs=4)
                    nc.tensor.matmul(
                        ps_s[:],
                        kT_loc[:, 0:2, ds(128 * mt, P)],
                        qT_c[:, 0:2, :],
                        start=True, stop=True, perf_mode=DR,
                    )
                    nc.scalar.activation(
                        E[:, mt, ds(512 * c, 512)], ps_s[:], AF.Exp,
                        scale=SEXP, accum_out=rowsum_parts[:, mt, c : c + 1],
                    )

            # ---- rowsums -> hn_scaled ----
            rs1 = small.tile([P, MT], f32, name="rs1", tag="rs1")
            nc.vector.tensor_reduce(rs1[:], rowsum_parts[:], AX.X, OP.add)
            nc.vector.reciprocal(recip_r[:], rs1[:])
            sch = small.tile([P, MT], f32, name="sch", tag="sch", bufs=1)
            nc.vector.tensor_tensor(
                sch[:], recip_r[:], M11.to_broadcast((P, MT)), OP.mult
            )
            nc.vector.memset(hn_scaled[:].bitcast(f32), 0.0)
            for mt in range(MT):
                nc.vector.tensor_scalar_mul(
                    hn_scaled[:, mt, 0:F], hn_loc[:, mt], sch[:, mt : mt + 1]
                )
                nc.vector.tensor_copy(
                    out=hn_scaled[:, mt, F : F + 1], in_=sch[:, mt : mt + 1]
                )

            # ---- partial = E.T @ hn_scaled -> bf16 -> DRAM (scalar stg) ----
            for g in range(8):
                stg = sc1.tile([P, 4, FP], bf16, name="stg", tag="stg", bufs=2)
                for k in range(4):
                    ic = 4 * g + k
                    ps_p = p1.tile([P, FP], f32, name="ps_p", tag="mm", bufs=4)
                    for jt in range(MT):
                        nc.tensor.matmul(
                            ps_p[:],
                            E[:, jt, ds(128 * ic, P)],
                            hn_scaled[:, jt, :],
                            start=(jt == 0), stop=(jt == MT - 1),
                        )
                    nc.scalar.activation(stg[:, k], ps_p[:], AF.Copy)
                nc.sync.dma_start(pd_v[:, ds(4 * g, 4), :], stg[:])

        # ET via 4 hardware DMA transposes (bf16), overlapping the collective
        for mt in range(MT):
            nc.sync.dma_start_transpose(ET_all[:, :, ds(128 * mt, P)], E[:, mt, :])

        nc.gpsimd.collective_compute(
            "ReduceScatter",
            mybir.AluOpType.add,
            replica_groups=[list(range(NCORES))],
            ins=[partial_dram[:]],
            outs=[rs_dram[:]],
        )
        nc.sync.dma_start(rs_sb[:], rs_dram.rearrange("(o p) f -> p o f", p=P))

        # ============ x phase ============
        with tc.tile_pool(name="p2", bufs=1, space="PSUM") as p2, \
             tc.tile_pool(name="sc2", bufs=1) as sc2:
            # L1 (vector has slack while partial streams out)
            for c in range(NCH):
                nc.vector.tensor_reduce(
                    l1_t[:, ds(4 * c, 4)], x_in[:, ds(4 * c, 4), :], AX.X, OP.add,
                    apply_absolute_value=True,
                )
            nc.vector.tensor_reduce(
                l1_t[:, ds(NT, MT)], x_lin[:], AX.X, OP.add,
                apply_absolute_value=True,
            )
            nc.vector.tensor_scalar_max(l1_t[:], l1_t[:], L1_EPS)
            nc.vector.reciprocal(rl1s_t[:], l1_t[:])
            nc.vector.tensor_scalar_mul(rl1s_t[:], rl1s_t[:], SA)

            # local rows: xn_loc_b (bf16), x8_l (fp8), diag, xnT_loc
            rl1p = small.tile([P, MT], f32, name="rl1p", tag="rl1p", bufs=1)
            nc.vector.tensor_scalar_mul(rl1p[:], rl1s_t[:, ds(NT, MT)], 1.0 / SA)
            sqj = sc2.tile([P, MT, F], f8, name="sqj")
            ssq = small.tile([P, MT], f32, name="ssq", tag="ssq", bufs=1)
            for j in range(MT):
                nc.vector.tensor_scalar_mul(
                    xn_loc_b[:, j], x_lin[:, j], rl1p[:, j : j + 1]
                )
                nc.scalar.activation(
                    sqj[:, j], x_lin[:, j], AF.Square,
                    accum_out=ssq[:, j : j + 1],
                )
            t0 = small.tile([P, MT], f32, name="t0d", tag="t0d", bufs=1)
            nc.vector.tensor_tensor(t0[:], rl1p[:], rl1p[:], OP.mult)
            nc.vector.tensor_tensor(diag[:], ssq[:], t0[:], OP.mult)
            x8_l = sc2.tile([P, MT, F], f8, name="x8_l")
            for j in range(MT):
                nc.vector.tensor_scalar_mul(
                    x8_l[:, j], x_lin[:, j], rl1s_t[:, NT + j : NT + j + 1]
                )
            for ft in range(FT):
                ps_t = p2.tile([P, R, 2], f8, name="ps_xtl", tag="tp", bufs=2)
                for j in range(MT):
                    nc.tensor.transpose(
                        ps_t[:, ts(j, P), 0], x8_l[:, j, ds(128 * ft, P)], ident8[:]
                    )
                nc.vector.tensor_copy(out=xnT_loc[:, ft], in_=ps_t[:, :, 0])

            # global: xh_b x-half (gpsimd), xnT (vector casts), Gram (DR)
            ps_G = p2.tile([P, 2 * F], f32, name="ps_G", tag="G", bufs=1)
            for c in range(NCH):
                for j in range(4):
                    nt = 4 * c + j
                    nc.scalar.activation(
                        xh_b[:, nt, 0:F], x_in[:, nt], AF.Copy,
                        scale=rl1s_t[:, nt : nt + 1],
                    )
                for ft in range(FT):
                    ps_t = p2.tile([P, R, 2], f8, name="ps_xt", tag="tp", bufs=2)
                    for j in range(4):
                        nt = 4 * c + j
                        nc.tensor.transpose(
                            ps_t[:, ts(j, P), 0],
                            xh_b[:, nt, ds(128 * ft, P)],
                            ident8[:],
                        )
                    nc.vector.tensor_copy(
                        out=xnT[:, ft, ds(512 * c, 512)], in_=ps_t[:, :, 0]
                    )
                for pr in range(2):
                    nt0 = 4 * c + 2 * pr
                    for m in range(FT):
                        nc.tensor.matmul(
                            ps_G[:, ts(m, F)],
                            xh_b[:, nt0 : nt0 + 2, ds(128 * m, P)],
                            xh_b[:, nt0 : nt0 + 2, 0:F],
                            start=(c == 0 and pr == 0),
                            stop=(c == NCH - 1 and pr == 1),
                            perf_mode=DR,
                        )
            for m in range(FT):
                nc.vector.tensor_scalar_mul(
                    G8[:, m], ps_G[:, ts(m, F)], SG / (SA * SA)
                )

        # ============ aggregation ============
        with tc.tile_pool(name="p3", bufs=1, space="PSUM") as p3, \
             tc.tile_pool(name="sc3", bufs=1) as sc3:
            ps_xs = [
                p3.tile([P, R], f32, name=f"ps_xs{mt}", tag=f"xs{mt}", bufs=1)
                for mt in range(MT)
            ]
            with tc.tile_pool(name="p3a", bufs=1, space="PSUM") as p3a:
                for icp in range(16):
                    axc2 = sc3.tile([P, 2, R], f8, name="axc2", tag="axc", bufs=3)
                    for q in range(2):
                        ic = 2 * icp + q
                        ps_ax = p3a.tile([P, R], f32, name="ps_ax", tag="ax",
                                         bufs=4)
                        nc.tensor.matmul(
                            ps_ax[:], xnT[:, 0:2, ds(128 * ic, P)],
                            xnT_loc[:, 0:2, :],
                            start=True, stop=True, perf_mode=DR,
                        )
                        eng = nc.scalar if q == 0 else nc.vector
                        if q == 0:
                            nc.scalar.activation(
                                axc2[:, q], ps_ax[:], AF.Copy,
                                scale=SA2 / (SA * SA),
                            )
                        else:
                            nc.vector.tensor_scalar_mul(
                                axc2[:, q], ps_ax[:], SA2 / (SA * SA)
                            )
                    for mt in range(MT):
                        nc.tensor.matmul(
                            ps_xs[mt][:],
                            axc2[:, 0:2, ds(128 * mt, P)],
                            xh_b[:, ds(2 * icp, 2), :],
                            start=(icp == 0), stop=(icp == 15), perf_mode=DR,
                        )

            # bslcT: (E @ xn).T accumulated f-major (fp8 lhsT x bf16 rhs)
            p3b_cm = tc.tile_pool(name="p3b", bufs=1, space="PSUM")
            p3b = p3b_cm.__enter__()
            ps_bT = [
                p3b.tile([P, R], f32, name=f"ps_bT{fs}", tag=f"bT{fs}", bufs=1)
                for fs in range(FT)
            ]
            for ic in range(NT):
                for fs in range(FT):
                    nc.tensor.matmul(
                        ps_bT[fs][:],
                        xh_b[:, ic, ds(128 * fs, P)],
                        ET_all[:, ic, :],
                        start=(ic == 0), stop=(ic == NT - 1),
                    )
            bT_sb = sc3.tile([P, FT, R], bf16, name="bT_sb")
            for fs in range(FT):
                nc.vector.tensor_copy(out=bT_sb[:, fs], in_=ps_bT[fs][:])

            # ---- x_out combine ----
            for mt in range(MT):
                ps_bb = p3b.tile([P, F], bf16, name="ps_bb", tag="w", bufs=2)
                for fs in range(FT):
                    nc.tensor.transpose(
                        ps_bb[:, ts(fs, P)], bT_sb[:, fs, ds(128 * mt, P)],
                        ident_b[:],
                    )
                xo = small.tile([P, F], f32, name="xo", tag="xo", bufs=2)
                nc.vector.tensor_scalar_mul(xo[:], ps_xs[mt][:, 0:F], m00s[:])
                scb = small.tile([P, 1], f32, name="scb", tag="scb")
                nc.vector.tensor_tensor(
                    scb[:], recip_r[:, mt : mt + 1], m10s[:], OP.mult
                )
                tb = small.tile([P, F], f32, name="tb", tag="wkb", bufs=2)
                nc.vector.tensor_scalar_mul(tb[:], ps_bb[:], scb[:])
                nc.vector.tensor_tensor(xo[:], xo[:], tb[:], OP.add)
                nc.vector.tensor_tensor(xo[:], xo[:], x_lin[:, mt], OP.add)
                nc.sync.dma_start(xout_v[:, mt], xo[:])

            # ---- late stats: srow / GX -> sumsq -> std ----
            s_f = small.tile([P, FT], f32, name="s_f", tag="s_f", bufs=1)
            nc.vector.tensor_reduce(s_f[:], xnT[:], AX.X, OP.add)
            s8 = small.tile([P, FT], f8, name="s8", tag="s8", bufs=1)
            nc.vector.tensor_scalar_mul(s8[:], s_f[:], SS)
            ps_sr = p3b.tile([P, MT], f32, name="ps_sr", tag="w", bufs=2)
            for mt in range(MT):
                for ft in range(FT):
                    nc.tensor.matmul(
                        ps_sr[:, mt : mt + 1],
                        xnT_loc[:, ft, ds(128 * mt, P)],
                        s8[:, ft : ft + 1],
                        start=(ft == 0), stop=(ft == 1),
                    )
            nc.vector.tensor_scalar_mul(srow[:], ps_sr[:], 1.0 / (SA * SA * SS))
            GX_sb = sc3.tile([P, FT, R], bf16, name="GX_sb")
            for fs in range(FT):
                ps_gx = p3b.tile([P, R], f32, name="ps_gx", tag="w", bufs=2)
                nc.tensor.matmul(
                    ps_gx[:], G8[:, 0:2, ds(128 * fs, P)], xnT_loc[:, 0:2, :],
                    start=True, stop=True, perf_mode=DR,
                )
                nc.vector.tensor_copy(out=GX_sb[:, fs], in_=ps_gx[:])
            for mt in range(MT):
                ps_gxt = p3b.tile([P, F], bf16, name="ps_gxt", tag="w", bufs=2)
                for fs in range(FT):
                    nc.tensor.transpose(
                        ps_gxt[:, ts(fs, P)], GX_sb[:, fs, ds(128 * mt, P)],
                        ident_b[:],
                    )
                nc.vector.tensor_copy(out=GXT[:, mt], in_=ps_gxt[:])
            sumsq = small.tile([P, MT], f32, name="sumsq", tag="sumsq", bufs=1)
            for mt in range(MT):
                tmp2 = small.tile([P, F], f32, name="tmp2", tag="wka", bufs=2)
                nc.vector.tensor_tensor(
                    tmp2[:], xn_loc_b[:, mt], GXT[:, mt], OP.mult
                )
                nc.vector.tensor_reduce(
                    sumsq[:, mt : mt + 1], tmp2[:], AX.X, OP.add
                )
            sr2 = small.tile([P, MT], f32, name="sr2", tag="sr2", bufs=1)
            nc.vector.scalar_tensor_tensor(
                sr2[:], srow[:], 1.0 / (N * (N - 1.0)), srow[:], OP.mult, OP.mult
            )
            t1 = small.tile([P, MT], f32, name="t1", tag="t1", bufs=1)
            nc.vector.scalar_tensor_tensor(
                t1[:], sumsq[:], 1.0 / (SG * SA * (N - 1.0)), sr2[:],
                OP.mult, OP.subtract,
            )
            # std = t1 * rsqrt(t1) (vector; avoids a scalar Sqrt table load)
            t1c = small.tile([P, MT], f32, name="t1c", tag="t1c", bufs=1)
            nc.vector.tensor_scalar_max(t1c[:], t1[:], 1e-30)
            rsq = newton_rsqrt(t1c[:], MT, "s")
            nc.vector.tensor_tensor(stdv[:], t1[:], rsq, OP.mult)

            # ---- h path ----
            h_agg_s = sc3.tile([P, MT, F], bf16, name="h_agg_s")
            for mt in range(MT):
                nc.vector.scalar_tensor_tensor(
                    h_agg_s[:, mt], ps_xs[mt][:, F : 2 * F], 1.0 / SA2,
                    rs_sb[:, mt, 0:F], OP.mult, OP.add,
                )
                nc.vector.tensor_copy(out=i_cols[:, mt, 0:1], in_=diag[:, mt : mt + 1])
                nc.vector.tensor_copy(out=i_cols[:, mt, 1:2], in_=srow[:, mt : mt + 1])
                nc.vector.tensor_copy(out=i_cols[:, mt, 2:3], in_=stdv[:, mt : mt + 1])
                c4 = small.tile([P, 1], f32, name="c4", tag="c4", bufs=4)
                nc.vector.tensor_tensor(c4[:], srow[:, mt : mt + 1], M01, OP.mult)
                nc.vector.tensor_tensor(
                    i_cols[:, mt, 3:4], rs_sb[:, mt, F : F + 1], c4[:], OP.add
                )
            nc.vector.memset(i_T[:].bitcast(f32), 0.0)
            for mt in range(MT):
                ps_i = p3b.tile([4, P], f32, name="ps_i", tag="w", bufs=2)
                nc.tensor.transpose(ps_i[:], i_cols[:, mt], ident_f[:])
                nc.vector.tensor_copy(out=i_T[:4, ds(128 * mt, P)], in_=ps_i[:])
            for ft in range(FT):
                ps_hat = p3b.tile([P, R], bf16, name="ps_hat", tag="w", bufs=2)
                for mt in range(MT):
                    nc.tensor.transpose(
                        ps_hat[:, ts(mt, P)], h_agg_s[:, mt, ds(128 * ft, P)],
                        ident_b[:],
                    )
                nc.scalar.activation(
                    h_aggT[:, ft], ps_hat[:], AF.Copy, scale=gam_f[:, ft]
                )
            for mt in range(MT):
                ps_h = p3b.tile([P, F], f32, name="ps_h", tag="w", bufs=2)
                for k in range(FT):
                    nc.tensor.matmul(
                        ps_h[:], h_aggT[:, k, ds(128 * mt, P)], wvT[:, k],
                        start=(k == 0), stop=False,
                    )
                nc.tensor.matmul(
                    ps_h[:], i_T[:, ds(128 * mt, P)], wvT3[:],
                    start=False, stop=True,
                )
                vmin = small.tile([P, F], f32, name="vmin", tag="wka", bufs=2)
                nc.vector.tensor_scalar_min(vmin[:], ps_h[:], 0.0)
                ev = small.tile([P, F], f32, name="ev", tag="wkb", bufs=2)
                nc.scalar.activation(ev[:], vmin[:], AF.Exp)
                vmax = small.tile([P, F], f32, name="vmax", tag="wka", bufs=2)
                nc.vector.tensor_scalar_max(vmax[:], ps_h[:], 0.0)
                ho = small.tile([P, F], f32, name="ho", tag="ho", bufs=2)
                nc.vector.tensor_tensor(ho[:], ev[:], vmax[:], OP.add)
                nc.vector.tensor_scalar_add(ho[:], ho[:], -1.0)
                nc.vector.tensor_tensor(ho[:], ho[:], h_lin[:, mt], OP.add)
                nc.sync.dma_start(hout_v[:, mt], ho[:])
            p3b_cm.__exit__(None, None, None)

    nc.finalize()
    return nc


def _make_in_maps(inputs):
    h = np.ascontiguousarray(inputs["h"], dtype=np.float32)
    x = np.ascontiguousarray(inputs["x"], dtype=np.float32)
    w_k = np.asarray(inputs["w_k"], np.float32)
    w_q = np.asarray(inputs["w_q"], np.float32)
    w_v = np.asarray(inputs["w_v"], np.float32)
    mixing = np.asarray(inputs["mixing"], np.float32)
    gam = np.ascontiguousarray(inputs["ln_gamma"], dtype=np.float32)
    bet = np.asarray(inputs["ln_beta"], np.float32)

    w_k8 = np.ascontiguousarray(w_k.T * WS)
    w_q8 = np.ascontiguousarray(w_q.T * WS)
    w_vT = np.ascontiguousarray(w_v.T)  # [F+3, F]
    w_vTm = np.ascontiguousarray(w_vT[:F])
    bvec = w_v[:, :F] @ bet  # [F]
    wv_tail = np.ascontiguousarray(
        np.concatenate([w_vT[F:], bvec[None, :]], axis=0)
    )
    me = np.exp(mixing)
    m = me / me.sum(axis=0, keepdims=True)
    m_n = np.ascontiguousarray(m.reshape(-1))  # [m00, m01, m10, m11]

    return [
        {
            "h": h,
            "x": x,
            "h_loc": np.ascontiguousarray(h[c * R : (c + 1) * R]),
            "x_loc": np.ascontiguousarray(x[c * R : (c + 1) * R]),
            "w_k8": w_k8,
            "w_q8": w_q8,
            "w_vTm": w_vTm,
            "wv_tail": wv_tail,
            "m_n": m_n,
            "ln_gamma": gam,
            "ln_beta": np.ascontiguousarray(bet),
        }
        for c in range(NCORES)
    ]


def kernel(h, x, w_k, w_q, w_v, mixing, ln_gamma, ln_beta):
    from concourse.bass_utils import run_bass_kernel_spmd

    if "nc" not in _CACHE:
        _CACHE["nc"] = _build()
    nc = _CACHE["nc"]

    in_maps = _make_in_maps(
        {
            "h": h,
            "x": x,
            "w_k": w_k,
            "w_q": w_q,
            "w_v": w_v,
            "mixing": mixing,
            "ln_gamma": ln_gamma,
            "ln_beta": ln_beta,
        }
    )
    res = run_bass_kernel_spmd(nc, in_maps, list(range(NCORES))).results
    h_out = np.concatenate([res[c]["h_out"] for c in range(NCORES)], axis=0)
    x_out = np.concatenate([res[c]["x_out"] for c in range(NCORES)], axis=0)
    return (h_out, x_out)


# revision 27
# speedup vs baseline: 1.4166x; 1.0394x over previous
# Trainium2 Bass kernel for the BronxLayer GNN message-passing problem (v2.1).
#
# Reference math (fp32):
#   hn = LayerNorm(h)*gamma + beta ; xn = x / max(|x|_1, 1e-12)
#   k = hn@w_k.T ; q = hn@w_q.T ; a_h = softmax(k@q.T/16) ; a_x = xn@xn.T
#   i = [diag(a_x), rowsum(a_x), rowstd(a_x, ddof=1)] ; m = softmax(mixing, 0)
#   x_out = (m00*a_x + m10*a_h)@xn + x
#   h_agg = m01*(a_x@hn) + m11*(a_h.T@hn)          (a_x symmetric)
#   h_out = elu([h_agg|i]@w_v.T) + h
#
# Sharding: nodes row-sharded over 8 cores.  The only cross-core term,
# m11*(a_h.T@hn), is reduced with one bf16 ReduceScatter of
# partial = E.T @ [hn*m11/rowsum | m11/rowsum], triggered right after the
# h-projection path so it overlaps the whole x/aggregation phase.
#
# Perf structure (v2.1):
#  - fp8e4+DoubleRow matmuls for qT/kT/S/a_xc/ps_xs/Gram/GX; E kept bf16 so
#    ET comes from 4 hardware DMA transposes instead of 64 PE transposes.
#  - per-chunk pipelined LayerNorm: bn_stats/bn_aggr + a bit-trick Newton
#    rsqrt on the vector engine (no scalar Sqrt -> scalar act table stays
#    on Exp the whole kernel, zero table reloads after the first).
#  - engine balance: applies on gpsimd, psum->sbuf casts split
#    scalar/vector, input DMA issue split across the two HWDGE queues.
import sys

if "/opt/trn_rl_repo" not in sys.path:
    sys.path.insert(0, "/opt/trn_rl_repo")

import numpy as np

N, F = 4096, 256
NCORES = 8
R = N // NCORES  # 512
P = 128
MT = R // P      # 4
NT = N // P      # 32
FT = F // P      # 2
NCH = N // 512   # 8
FP = 272         # partial width: F cols + colsum col + pad (16B-aligned)
LN_EPS = 1e-5
L1_EPS = 1e-12
WS = 8.0                     # w_k / w_q prescale (host)
SEXP = (1.0 / 16.0) / (WS * WS)
SA = 256.0                   # xn fp8 scale
SA2 = 8192.0                 # a_x fp8 scale (psum SA^2*a_x -> copy scale 1/8)
SG = 1024.0                  # Gram fp8 scale
SS = 0.25                    # colsum fp8 scale relative to SA*colsum
RSQRT_MAGIC = 0x5F3759DF

_CACHE = {}


def _build():
    import contextlib

    import concourse.mybir as mybir
    import concourse.tile as tile
    from concourse import bacc
    from concourse.bass import ds, ts
    from concourse.masks import make_identity

    f32 = mybir.dt.float32
    f32r = mybir.dt.float32r
    bf16 = mybir.dt.bfloat16
    f8 = mybir.dt.float8e4
    i32 = mybir.dt.int32
    AF = mybir.ActivationFunctionType
    OP = mybir.AluOpType
    AX = mybir.AxisListType
    DR = mybir.MatmulPerfMode.DoubleRow

    nc = bacc.Bacc(None, num_devices=NCORES)

    h_ext = nc.declare_dram_parameter("h", [N, F], f32, isOutput=False)
    x_ext = nc.declare_dram_parameter("x", [N, F], f32, isOutput=False)
    hloc_ext = nc.declare_dram_parameter("h_loc", [R, F], f32, isOutput=False)
    xloc_ext = nc.declare_dram_parameter("x_loc", [R, F], f32, isOutput=False)
    wk_ext = nc.declare_dram_parameter("w_k8", [F, F], f32, isOutput=False)
    wq_ext = nc.declare_dram_parameter("w_q8", [F, F], f32, isOutput=False)
    wv_ext = nc.declare_dram_parameter("w_vTm", [F, F], f32r, isOutput=False)
    wvt_ext = nc.declare_dram_parameter("wv_tail", [4, F], f32r, isOutput=False)
    mn_ext = nc.declare_dram_parameter("m_n", [4], f32, isOutput=False)
    gam_ext = nc.declare_dram_parameter("ln_gamma", [F], f32, isOutput=False)
    bet_ext = nc.declare_dram_parameter("ln_beta", [F], f32, isOutput=False)
    hout_ext = nc.declare_dram_parameter("h_out", [R, F], f32, isOutput=True)
    xout_ext = nc.declare_dram_parameter("x_out", [R, F], f32, isOutput=True)

    h_v = h_ext.rearrange("(o p) f -> p o f", p=P)
    x_v = x_ext.rearrange("(o p) f -> p o f", p=P)
    hloc_v = hloc_ext.rearrange("(o p) f -> p o f", p=P)
    xloc_v = xloc_ext.rearrange("(o p) f -> p o f", p=P)
    hout_v = hout_ext.rearrange("(o p) f -> p o f", p=P)
    xout_v = xout_ext.rearrange("(o p) f -> p o f", p=P)

    NS = NT + MT

    with tile.TileContext(nc) as tc, contextlib.ExitStack() as ctx:
        const = ctx.enter_context(tc.tile_pool(name="const", bufs=1))
        persist = ctx.enter_context(tc.tile_pool(name="persist", bufs=1))
        dram = ctx.enter_context(tc.tile_pool(name="dram", bufs=1, space="DRAM"))
        small = ctx.enter_context(tc.tile_pool(name="small", bufs=3))

        # ---------------- persistent tensors ----------------
        h_in = persist.tile([P, NT, F], f32, name="h_in")
        x_in = persist.tile([P, NT, F], f32, name="x_in")
        h_lin = persist.tile([P, MT, F], f32, name="h_lin")
        x_lin = persist.tile([P, MT, F], f32, name="x_lin")
        xh_b = persist.tile([P, NT, 2 * F], f8, name="xh_b")
        xnT = persist.tile([P, FT, N], f8, name="xnT")
        kT_loc = persist.tile([P, FT, R], f8, name="kT_loc")
        xnT_loc = persist.tile([P, FT, R], f8, name="xnT_loc")
        E = persist.tile([P, MT, N], bf16, name="E")
        ET_all = persist.tile([P, NT, R], bf16, name="ET_all")
        hn_loc = persist.tile([P, MT, F], bf16, name="hn_loc")
        xn_loc_b = persist.tile([P, MT, F], bf16, name="xn_loc_b")
        hn_scaled = persist.tile([P, MT, FP], bf16, name="hn_scaled")
        rowsum_parts = persist.tile([P, MT, NCH], f32, name="rowsum_parts")
        recip_r = persist.tile([P, MT], f32, name="recip_r")
        diag = persist.tile([P, MT], f32, name="diag")
        srow = persist.tile([P, MT], f32, name="srow")
        stdv = persist.tile([P, MT], f32, name="stdv")
        G8 = persist.tile([P, FT, F], f8, name="G8")
        GXT = persist.tile([P, MT, F], bf16, name="GXT")
        i_cols = persist.tile([P, MT, 4], f32, name="i_cols")
        i_T = persist.tile([P, R], f32r, name="i_T")
        rs_sb = persist.tile([P, MT, FP], bf16, name="rs_sb")
        h_aggT = persist.tile([P, FT, R], f32r, name="h_aggT")
        l1_t = persist.tile([P, NS], f32, name="l1_t")
        rl1s_t = persist.tile([P, NS], f32, name="rl1s_t")

        partial_dram = dram.tile([N, FP], bf16, name="partial_dram")
        rs_dram = dram.tile([R, FP], bf16, name="rs_dram")
        pd_v = partial_dram.rearrange("(a p) f -> p a f", p=P)

        # ---------------- DMA issue: h on sync, x/weights on scalar --------
        nc.sync.dma_start(h_lin[:], hloc_v[:])
        for c in range(NCH):
            nc.sync.dma_start(h_in[:, ds(4 * c, 4), :], h_v[:, ds(4 * c, 4)])

        m_bc = const.tile([P, 4], f32, name="m_bc")
        nc.sync.dma_start(m_bc[:], mn_ext.rearrange("a -> () a").to_broadcast((P, 4)))
        M00, M01, M10, M11 = (m_bc[:, j : j + 1] for j in range(4))
        gam_f = const.tile([P, FT, 1], f32, name="gam_f")
        nc.sync.dma_start(gam_f[:, :, 0], gam_ext.rearrange("(o p) -> p o", p=P))
        bet_f = const.tile([P, FT, 1], f32, name="bet_f")
        nc.sync.dma_start(bet_f[:, :, 0], bet_ext.rearrange("(o p) -> p o", p=P))
        wvT = const.tile([P, FT, F], f32r, name="wvT")
        nc.scalar.dma_start(wvT[:], wv_ext.rearrange("(o p) f -> p o f", p=P))
        wvT3 = const.tile([P, F], f32r, name="wvT3")
        nc.vector.memset(wvT3[:].bitcast(f32), 0.0)
        nc.scalar.dma_start(wvT3[:4, :], wvt_ext[:])

        ident8 = const.tile([P, P], f8, name="ident8")
        make_identity(nc, ident8)
        ident_b = const.tile([P, P], bf16, name="ident_b")
        make_identity(nc, ident_b)
        ident_f = const.tile([P, P], f32, name="ident_f")
        make_identity(nc, ident_f)

        wq8 = const.tile([P, FT, F], f8, name="wq8")
        wk8 = const.tile([P, FT, F], f8, name="wk8")

        rm01 = const.tile([P, 1], f32, name="rm01")
        nc.vector.reciprocal(rm01[:], M01)
        gam_eff = const.tile([P, FT, 1], f32, name="gam_eff")
        for ft in range(FT):
            nc.vector.tensor_tensor(gam_eff[:, ft], gam_f[:, ft], rm01[:], OP.mult)
        m00s = const.tile([P, 1], f32, name="m00s")
        nc.vector.tensor_scalar_mul(m00s[:], M00, 1.0 / (SA2 * SA))
        m10s = const.tile([P, 1], f32, name="m10s")
        nc.vector.tensor_scalar_mul(m10s[:], M10, 1.0 / SA)

        def newton_rsqrt(ve, nrows, tag):
            """1/sqrt(ve) on the vector engine (ve: [P, nrows] f32 AP,
            destroyed).  Returns an f32 AP view.  ~7 small DVE ops."""
            ish = small.tile([P, nrows], i32, name="nr_i" + tag, tag="nri" + tag)
            nc.vector.tensor_scalar(
                ish[:], ve.bitcast(i32), scalar1=1, scalar2=None,
                op0=OP.logical_shift_right,
            )
            y0 = small.tile([P, nrows], i32, name="nr_y" + tag, tag="nry" + tag)
            nc.vector.tensor_scalar(
                y0[:], ish[:], scalar1=-1, scalar2=RSQRT_MAGIC, op0=OP.mult,
                op1=OP.add,
            )
            yf = y0[:].bitcast(f32)
            hv = small.tile([P, nrows], f32, name="nr_h" + tag, tag="nrh" + tag)
            nc.vector.tensor_scalar_mul(hv[:], ve, 0.5)
            t = small.tile([P, nrows], f32, name="nr_t" + tag, tag="nrt" + tag)
            for _ in range(2):
                nc.vector.tensor_tensor(t[:], yf, yf, OP.mult)
                nc.vector.tensor_tensor(t[:], t[:], hv[:], OP.mult)
                nc.vector.tensor_scalar(
                    t[:], t[:], scalar1=-1.0, scalar2=1.5, op0=OP.mult, op1=OP.add
                )
                nc.vector.tensor_tensor(yf, yf, t[:], OP.mult)
            return yf

        def ln_coeffs(src_blk, nrows, tag):
            """bn_stats -> (rstm, nmrm) = (m01/sd, -mean*m01/sd) and
            (rstd, nmr) plain views for the local tile."""
            st6 = small.tile([P, nrows, 6], f32, name="st6" + tag, tag="st6")
            for j in range(nrows):
                nc.vector.bn_stats(st6[:, j], src_blk[:, j])
            mv = small.tile([P, nrows, 2], f32, name="mv" + tag, tag="mv")
            for j in range(nrows):
                nc.vector.bn_aggr(mv[:, j], st6[:, j])
            ve = small.tile([P, nrows], f32, name="ve" + tag, tag="ve")
            nc.vector.tensor_scalar_add(ve[:], mv[:, :, 1], LN_EPS)
            rstd = newton_rsqrt(ve[:], nrows, tag)
            rstm = small.tile([P, nrows], f32, name="rsm" + tag, tag="rsm")
            nc.vector.tensor_tensor(
                rstm[:], rstd, M01.to_broadcast((P, nrows)), OP.mult
            )
            nmrm = small.tile([P, nrows], f32, name="nmm" + tag, tag="nmm")
            nc.vector.scalar_tensor_tensor(
                nmrm[:], mv[:, :, 0], -1.0, rstm[:], OP.mult, OP.mult
            )
            return rstd, mv, rstm, nmrm

        # ============ h phase ============
        with tc.tile_pool(name="p1", bufs=1, space="PSUM") as p1, \
             tc.tile_pool(name="sc1", bufs=1) as sc1:
            # weight staging + fp8 conversion (gpsimd; vector stays free)
            wq_st = sc1.tile([P, FT, F], f32, name="wq_st", tag="wst", bufs=1)
            nc.sync.dma_start(wq_st[:], wq_ext.rearrange("(o p) f -> p o f", p=P))
            nc.vector.tensor_copy(out=wq8[:], in_=wq_st[:])
            wk_st = sc1.tile([P, FT, F], f32, name="wk_st", tag="wst", bufs=1)
            nc.sync.dma_start(wk_st[:], wk_ext.rearrange("(o p) f -> p o f", p=P))
            nc.vector.tensor_copy(out=wk8[:], in_=wk_st[:])
            # ---- local tile: hn_loc, lh8, hnT_l, kT_loc ----
            rstd_l, mv_l, rstm_l, nmrm_l = ln_coeffs(h_lin[:], MT, "l")
            lh8 = sc1.tile([P, MT, F], f8, name="lh8")
            for j in range(MT):
                nc.vector.tensor_scalar(
                    lh8[:, j], h_lin[:, j],
                    scalar1=rstm_l[:, j : j + 1], scalar2=nmrm_l[:, j : j + 1],
                    op0=OP.mult, op1=OP.add,
                )
            nmr_l = small.tile([P, MT], f32, name="nmr_l", tag="nmrl", bufs=1)
            nc.vector.scalar_tensor_tensor(
                nmr_l[:], mv_l[:, :, 0], -1.0, rstd_l, OP.mult, OP.mult
            )
            for j in range(MT):
                nc.vector.tensor_scalar(
                    hn_loc[:, j], h_lin[:, j],
                    scalar1=rstd_l[:, j : j + 1], scalar2=nmr_l[:, j : j + 1],
                    op0=OP.mult, op1=OP.add,
                )
            hnT_l = sc1.tile([P, FT, R], f8, name="hnT_l")
            for ft in range(FT):
                ps_t = p1.tile([P, R, 2], f8, name="ps_tl", tag="tp", bufs=2)
                for j in range(MT):
                    nc.tensor.transpose(
                        ps_t[:, ts(j, P), 0], lh8[:, j, ds(128 * ft, P)], ident8[:]
                    )
                nc.scalar.activation(
                    hnT_l[:, ft], ps_t[:, :, 0], AF.Identity,
                    bias=bet_f[:, ft], scale=gam_eff[:, ft],
                )
            for fo in range(FT):
                ps_k = p1.tile([P, R], f32, name="ps_k", tag="mm", bufs=4)
                nc.tensor.matmul(
                    ps_k[:], wk8[:, 0:2, ds(128 * fo, P)], hnT_l[:, 0:2, :],
                    start=True, stop=True, perf_mode=DR,
                )
                nc.scalar.activation(kT_loc[:, fo], ps_k[:], AF.Copy)

            # ---- global chunks, fully pipelined ----
            for c in range(NCH):
                blk = h_in[:, ds(4 * c, 4), :]
                _, _, rstm, nmrm = ln_coeffs(blk, 4, "c")
                for j in range(4):
                    nt = 4 * c + j
                    nc.vector.tensor_scalar(
                        xh_b[:, nt, F : 2 * F], h_in[:, nt],
                        scalar1=rstm[:, j : j + 1], scalar2=nmrm[:, j : j + 1],
                        op0=OP.mult, op1=OP.add,
                    )
                hnT_c = sc1.tile([P, FT, R], f8, name="hnT_c", tag="hnT", bufs=2)
                for ft in range(FT):
                    ps_t = p1.tile([P, R, 2], f8, name="ps_t", tag="tp", bufs=2)
                    for j in range(4):
                        nt = 4 * c + j
                        nc.tensor.transpose(
                            ps_t[:, ts(j, P), 0],
                            xh_b[:, nt, ds(F + 128 * ft, P)],
                            ident8[:],
                        )
                    nc.vector.tensor_scalar(
                        hnT_c[:, ft], ps_t[:, :, 0],
                        scalar1=gam_eff[:, ft], scalar2=bet_f[:, ft],
                        op0=OP.mult, op1=OP.add,
                    )
                qT_c = sc1.tile([P, FT, R], f8, name="qT_c", tag="qTc", bufs=2)
                for fo in range(FT):
                    ps_q = p1.tile([P, R], f32, name="ps_q", tag="mm", bufs=4)
                    nc.tensor.matmul(
                        ps_q[:], wq8[:, 0:2, ds(128 * fo, P)], hnT_c[:, 0:2, :],
                        start=True, stop=True, perf_mode=DR,
                    )
                    nc.scalar.activation(qT_c[:, fo], ps_q[:], AF.Copy)
                for mt in range(MT):
                    ps_s = p1.tile([P, R], f32, name="ps_s", tag="mm", bufs=4)
                    nc.tensor.matmul(
                        ps_s[:],
                        kT_loc[:, 0:2, ds(128 * mt, P)],
                        qT_c[:, 0:2, :],
                        start=True, stop=True, perf_mode=DR,
                    )
                    nc.scalar.activation(
                        E[:, mt, ds(512 * c, 512)], ps_s[:], AF.Exp,
                        scale=SEXP, accum_out=rowsum_parts[:, mt, c : c + 1],
                    )

            # x inputs stream in after h owns the early HBM bandwidth
            nc.scalar.dma_start(x_lin[:], xloc_v[:])
            for c in range(NCH):
                nc.scalar.dma_start(
                    x_in[:, ds(4 * c, 4), :], x_v[:, ds(4 * c, 4)]
                )

            # ---- rowsums -> hn_scaled ----
            rs1 = small.tile([P, MT], f32, name="rs1", tag="rs1")
            nc.vector.tensor_reduce(rs1[:], rowsum_parts[:], AX.X, OP.add)
            nc.vector.reciprocal(recip_r[:], rs1[:])
            sch = small.tile([P, MT], f32, name="sch", tag="sch", bufs=1)
            nc.vector.tensor_tensor(
                sch[:], recip_r[:], M11.to_broadcast((P, MT)), OP.mult
            )
            nc.vector.memset(hn_scaled[:].bitcast(f32), 0.0)
            for mt in range(MT):
                nc.vector.tensor_scalar_mul(
                    hn_scaled[:, mt, 0:F], hn_loc[:, mt], sch[:, mt : mt + 1]
                )
                nc.vector.tensor_copy(
                    out=hn_scaled[:, mt, F : F + 1], in_=sch[:, mt : mt + 1]
                )

            # ---- partial = E.T @ hn_scaled -> bf16 -> DRAM (scalar stg) ----
            for g in range(8):
                stg = sc1.tile([P, 4, FP], bf16, name="stg", tag="stg", bufs=2)
                for k in range(4):
                    ic = 4 * g + k
                    ps_p = p1.tile([P, FP], f32, name="ps_p", tag="mm", bufs=4)
                    for jt in range(MT):
                        nc.tensor.matmul(
                            ps_p[:],
                            E[:, jt, ds(128 * ic, P)],
                            hn_scaled[:, jt, :],
                            start=(jt == 0), stop=(jt == MT - 1),
                        )
                    nc.scalar.activation(stg[:, k], ps_p[:], AF.Copy)
                nc.sync.dma_start(pd_v[:, ds(4 * g, 4), :], stg[:])

        # ET via 4 hardware DMA transposes (bf16), overlapping the collective
        for mt in range(MT):
            nc.sync.dma_start_transpose(ET_all[:, :, ds(128 * mt, P)], E[:, mt, :])

        nc.gpsimd.collective_compute(
            "ReduceScatter",
            mybir.AluOpType.add,
            replica_groups=[list(range(NCORES))],
            ins=[partial_dram[:]],
            outs=[rs_dram[:]],
        )
        nc.sync.dma_start(rs_sb[:], rs_dram.rearrange("(o p) f -> p o f", p=P))

        # ============ x phase ============
        with tc.tile_pool(name="p2", bufs=1, space="PSUM") as p2, \
             tc.tile_pool(name="sc2", bufs=1) as sc2:
            # L1 (vector has slack while partial streams out)
            for c in range(NCH):
                nc.vector.tensor_reduce(
                    l1_t[:, ds(4 * c, 4)], x_in[:, ds(4 * c, 4), :], AX.X, OP.add,
                    apply_absolute_value=True,
                )
            nc.vector.tensor_reduce(
                l1_t[:, ds(NT, MT)], x_lin[:], AX.X, OP.add,
                apply_absolute_value=True,
            )
            nc.vector.tensor_scalar_max(l1_t[:], l1_t[:], L1_EPS)
            nc.vector.reciprocal(rl1s_t[:], l1_t[:])
            nc.vector.tensor_scalar_mul(rl1s_t[:], rl1s_t[:], SA)

            # local rows: xn_loc_b (bf16), x8_l (fp8), diag, xnT_loc
            rl1p = small.tile([P, MT], f32, name="rl1p", tag="rl1p", bufs=1)
            nc.vector.tensor_scalar_mul(rl1p[:], rl1s_t[:, ds(NT, MT)], 1.0 / SA)
            sqj = sc2.tile([P, MT, F], f8, name="sqj")
            ssq = small.tile([P, MT], f32, name="ssq", tag="ssq", bufs=1)
            for j in range(MT):
                nc.vector.tensor_scalar_mul(
                    xn_loc_b[:, j], x_lin[:, j], rl1p[:, j : j + 1]
                )
                nc.scalar.activation(
                    sqj[:, j], x_lin[:, j], AF.Square,
                    accum_out=ssq[:, j : j + 1],
                )
            t0 = small.tile([P, MT], f32, name="t0d", tag="t0d", bufs=1)
            nc.vector.tensor_tensor(t0[:], rl1p[:], rl1p[:], OP.mult)
            nc.vector.tensor_tensor(diag[:], ssq[:], t0[:], OP.mult)
            x8_l = sc2.tile([P, MT, F], f8, name="x8_l")
            for j in range(MT):
                nc.vector.tensor_scalar_mul(
                    x8_l[:, j], x_lin[:, j], rl1s_t[:, NT + j : NT + j + 1]
                )
            for ft in range(FT):
                ps_t = p2.tile([P, R, 2], f8, name="ps_xtl", tag="tp", bufs=2)
                for j in range(MT):
                    nc.tensor.transpose(
                        ps_t[:, ts(j, P), 0], x8_l[:, j, ds(128 * ft, P)], ident8[:]
                    )
                nc.vector.tensor_copy(out=xnT_loc[:, ft], in_=ps_t[:, :, 0])

            # global: xh_b x-half (gpsimd), xnT (vector casts), Gram (DR)
            ps_G = p2.tile([P, 2 * F], f32, name="ps_G", tag="G", bufs=1)
            for c in range(NCH):
                for j in range(4):
                    nt = 4 * c + j
                    nc.vector.tensor_scalar_mul(
                        xh_b[:, nt, 0:F], x_in[:, nt], rl1s_t[:, nt : nt + 1]
                    )
                for ft in range(FT):
                    ps_t = p2.tile([P, R, 2], f8, name="ps_xt", tag="tp", bufs=2)
                    for j in range(4):
                        nt = 4 * c + j
                        nc.tensor.transpose(
                            ps_t[:, ts(j, P), 0],
                            xh_b[:, nt, ds(128 * ft, P)],
                            ident8[:],
                        )
                    nc.vector.tensor_copy(
                        out=xnT[:, ft, ds(512 * c, 512)], in_=ps_t[:, :, 0]
                    )
                for pr in range(2):
                    nt0 = 4 * c + 2 * pr
                    for m in range(FT):
                        nc.tensor.matmul(
                            ps_G[:, ts(m, F)],
                            xh_b[:, nt0 : nt0 + 2, ds(128 * m, P)],
                            xh_b[:, nt0 : nt0 + 2, 0:F],
                            start=(c == 0 and pr == 0),
                            stop=(c == NCH - 1 and pr == 1),
                            perf_mode=DR,
                        )
            for m in range(FT):
                nc.vector.tensor_scalar_mul(
                    G8[:, m], ps_G[:, ts(m, F)], SG / (SA * SA)
                )

        # ============ aggregation ============
        with tc.tile_pool(name="p3", bufs=1, space="PSUM") as p3, \
             tc.tile_pool(name="sc3", bufs=1) as sc3:
            ps_xs = [
                p3.tile([P, R], f32, name=f"ps_xs{mt}", tag=f"xs{mt}", bufs=1)
                for mt in range(MT)
            ]
            with tc.tile_pool(name="p3a", bufs=1, space="PSUM") as p3a:
                for icp in range(16):
                    axc2 = sc3.tile([P, 2, R], f8, name="axc2", tag="axc", bufs=3)
                    for q in range(2):
                        ic = 2 * icp + q
                        ps_ax = p3a.tile([P, R], f32, name="ps_ax", tag="ax",
                                         bufs=4)
                        nc.tensor.matmul(
                            ps_ax[:], xnT[:, 0:2, ds(128 * ic, P)],
                            xnT_loc[:, 0:2, :],
                            start=True, stop=True, perf_mode=DR,
                        )
                        eng = nc.scalar if q == 0 else nc.vector
                        if q == 0:
                            nc.scalar.activation(
                                axc2[:, q], ps_ax[:], AF.Copy,
                                scale=SA2 / (SA * SA),
                            )
                        else:
                            nc.vector.tensor_scalar_mul(
                                axc2[:, q], ps_ax[:], SA2 / (SA * SA)
                            )
                    for mt in range(MT):
                        nc.tensor.matmul(
                            ps_xs[mt][:],
                            axc2[:, 0:2, ds(128 * mt, P)],
                            xh_b[:, ds(2 * icp, 2), :],
                            start=(icp == 0), stop=(icp == 15), perf_mode=DR,
                        )

            # bslcT: (E @ xn).T accumulated f-major (fp8 lhsT x bf16 rhs)
            p3b_cm = tc.tile_pool(name="p3b", bufs=1, space="PSUM")
            p3b = p3b_cm.__enter__()
            ps_bT = [
                p3b.tile([P, R], f32, name=f"ps_bT{fs}", tag=f"bT{fs}", bufs=1)
                for fs in range(FT)
            ]
            for ic in range(NT):
                for fs in range(FT):
                    nc.tensor.matmul(
                        ps_bT[fs][:],
                        xh_b[:, ic, ds(128 * fs, P)],
                        ET_all[:, ic, :],
                        start=(ic == 0), stop=(ic == NT - 1),
                    )
            bT_sb = sc3.tile([P, FT, R], bf16, name="bT_sb")
            for fs in range(FT):
                nc.vector.tensor_copy(out=bT_sb[:, fs], in_=ps_bT[fs][:])

            # ---- x_out combine ----
            for mt in range(MT):
                ps_bb = p3b.tile([P, F], bf16, name="ps_bb", tag="w", bufs=2)
                for fs in range(FT):
                    nc.tensor.transpose(
                        ps_bb[:, ts(fs, P)], bT_sb[:, fs, ds(128 * mt, P)],
                        ident_b[:],
                    )
                xo = small.tile([P, F], f32, name="xo", tag="xo", bufs=2)
                nc.vector.tensor_scalar_mul(xo[:], ps_xs[mt][:, 0:F], m00s[:])
                scb = small.tile([P, 1], f32, name="scb", tag="scb")
                nc.vector.tensor_tensor(
                    scb[:], recip_r[:, mt : mt + 1], m10s[:], OP.mult
                )
                tb = small.tile([P, F], f32, name="tb", tag="wkb", bufs=2)
                nc.vector.tensor_scalar_mul(tb[:], ps_bb[:], scb[:])
                nc.vector.tensor_tensor(xo[:], xo[:], tb[:], OP.add)
                nc.vector.tensor_tensor(xo[:], xo[:], x_lin[:, mt], OP.add)
                nc.sync.dma_start(xout_v[:, mt], xo[:])

            # ---- late stats: srow / GX -> sumsq -> std ----
            s_f = small.tile([P, FT], f32, name="s_f", tag="s_f", bufs=1)
            nc.vector.tensor_reduce(s_f[:], xnT[:], AX.X, OP.add)
            s8 = small.tile([P, FT], f8, name="s8", tag="s8", bufs=1)
            nc.vector.tensor_scalar_mul(s8[:], s_f[:], SS)
            ps_sr = p3b.tile([P, MT], f32, name="ps_sr", tag="w", bufs=2)
            for mt in range(MT):
                for ft in range(FT):
                    nc.tensor.matmul(
                        ps_sr[:, mt : mt + 1],
                        xnT_loc[:, ft, ds(128 * mt, P)],
                        s8[:, ft : ft + 1],
                        start=(ft == 0), stop=(ft == 1),
                    )
            nc.vector.tensor_scalar_mul(srow[:], ps_sr[:], 1.0 / (SA * SA * SS))
            GX_sb = sc3.tile([P, FT, R], bf16, name="GX_sb")
            for fs in range(FT):
                ps_gx = p3b.tile([P, R], f32, name="ps_gx", tag="w", bufs=2)
                nc.tensor.matmul(
                    ps_gx[:], G8[:, 0:2, ds(128 * fs, P)], xnT_loc[:, 0:2, :],
                    start=True, stop=True, perf_mode=DR,
                )
                nc.vector.tensor_copy(out=GX_sb[:, fs], in_=ps_gx[:])
            for mt in range(MT):
                ps_gxt = p3b.tile([P, F], bf16, name="ps_gxt", tag="w", bufs=2)
                for fs in range(FT):
                    nc.tensor.transpose(
                        ps_gxt[:, ts(fs, P)], GX_sb[:, fs, ds(128 * mt, P)],
                        ident_b[:],
                    )
                nc.vector.tensor_copy(out=GXT[:, mt], in_=ps_gxt[:])
            sumsq = small.tile([P, MT], f32, name="sumsq", tag="sumsq", bufs=1)
            for mt in range(MT):
                tmp2 = small.tile([P, F], f32, name="tmp2", tag="wka", bufs=2)
                nc.vector.tensor_tensor(
                    tmp2[:], xn_loc_b[:, mt], GXT[:, mt], OP.mult
                )
                nc.vector.tensor_reduce(
                    sumsq[:, mt : mt + 1], tmp2[:], AX.X, OP.add
                )
            sr2 = small.tile([P, MT], f32, name="sr2", tag="sr2", bufs=1)
            nc.vector.scalar_tensor_tensor(
                sr2[:], srow[:], 1.0 / (N * (N - 1.0)), srow[:], OP.mult, OP.mult
            )
            t1 = small.tile([P, MT], f32, name="t1", tag="t1", bufs=1)
            nc.vector.scalar_tensor_tensor(
                t1[:], sumsq[:], 1.0 / (SG * SA * (N - 1.0)), sr2[:],
                OP.mult, OP.subtract,
            )
            # std = t1 * rsqrt(t1) (vector; avoids a scalar Sqrt table load)
            t1c = small.tile([P, MT], f32, name="t1c", tag="t1c", bufs=1)
            nc.vector.tensor_scalar_max(t1c[:], t1[:], 1e-30)
            rsq = newton_rsqrt(t1c[:], MT, "s")
            nc.vector.tensor_tensor(stdv[:], t1[:], rsq, OP.mult)

            # ---- h path ----
            h_agg_s = sc3.tile([P, MT, F], bf16, name="h_agg_s")
            for mt in range(MT):
                nc.vector.scalar_tensor_tensor(
                    h_agg_s[:, mt], ps_xs[mt][:, F : 2 * F], 1.0 / SA2,
                    rs_sb[:, mt, 0:F], OP.mult, OP.add,
                )
                nc.vector.tensor_copy(out=i_cols[:, mt, 0:1], in_=diag[:, mt : mt + 1])
                nc.vector.tensor_copy(out=i_cols[:, mt, 1:2], in_=srow[:, mt : mt + 1])
                nc.vector.tensor_copy(out=i_cols[:, mt, 2:3], in_=stdv[:, mt : mt + 1])
                c4 = small.tile([P, 1], f32, name="c4", tag="c4", bufs=4)
                nc.vector.tensor_tensor(c4[:], srow[:, mt : mt + 1], M01, OP.mult)
                nc.vector.tensor_tensor(
                    i_cols[:, mt, 3:4], rs_sb[:, mt, F : F + 1], c4[:], OP.add
                )
            nc.vector.memset(i_T[:].bitcast(f32), 0.0)
            for mt in range(MT):
                ps_i = p3b.tile([4, P], f32, name="ps_i", tag="w", bufs=2)
                nc.tensor.transpose(ps_i[:], i_cols[:, mt], ident_f[:])
                nc.vector.tensor_copy(out=i_T[:4, ds(128 * mt, P)], in_=ps_i[:])
            for ft in range(FT):
                ps_hat = p3b.tile([P, R], bf16, name="ps_hat", tag="w", bufs=2)
                for mt in range(MT):
                    nc.tensor.transpose(
                        ps_hat[:, ts(mt, P)], h_agg_s[:, mt, ds(128 * ft, P)],
                        ident_b[:],
                    )
                nc.scalar.activation(
                    h_aggT[:, ft], ps_hat[:], AF.Copy, scale=gam_f[:, ft]
                )
            for mt in range(MT):
                ps_h = p3b.tile([P, F], f32, name="ps_h", tag="w", bufs=2)
                for k in range(FT):
                    nc.tensor.matmul(
                        ps_h[:], h_aggT[:, k, ds(128 * mt, P)], wvT[:, k],
                        start=(k == 0), stop=False,
                    )
                nc.tensor.matmul(
                    ps_h[:], i_T[:, ds(128 * mt, P)], wvT3[:],
                    start=False, stop=True,
                )
                vmin = small.tile([P, F], f32, name="vmin", tag="wka", bufs=2)
                nc.vector.tensor_scalar_min(vmin[:], ps_h[:], 0.0)
                ev = small.tile([P, F], f32, name="ev", tag="wkb", bufs=2)
                nc.scalar.activation(ev[:], vmin[:], AF.Exp)
                vmax = small.tile([P, F], f32, name="vmax", tag="wka", bufs=2)
                nc.vector.tensor_scalar_max(vmax[:], ps_h[:], 0.0)
                ho = small.tile([P, F], f32, name="ho", tag="ho", bufs=2)
                nc.vector.tensor_tensor(ho[:], ev[:], vmax[:], OP.add)
                nc.vector.tensor_scalar_add(ho[:], ho[:], -1.0)
                nc.vector.tensor_tensor(ho[:], ho[:], h_lin[:, mt], OP.add)
                nc.sync.dma_start(hout_v[:, mt], ho[:])
            p3b_cm.__exit__(None, None, None)

    nc.finalize()
    return nc


def _make_in_maps(inputs):
    h = np.ascontiguousarray(inputs["h"], dtype=np.float32)
    x = np.ascontiguousarray(inputs["x"], dtype=np.float32)
    w_k = np.asarray(inputs["w_k"], np.float32)
    w_q = np.asarray(inputs["w_q"], np.float32)
    w_v = np.asarray(inputs["w_v"], np.float32)
    mixing = np.asarray(inputs["mixing"], np.float32)
    gam = np.ascontiguousarray(inputs["ln_gamma"], dtype=np.float32)
    bet = np.asarray(inputs["ln_beta"], np.float32)

    w_k8 = np.ascontiguousarray(w_k.T * WS)
    w_q8 = np.ascontiguousarray(w_q.T * WS)
    w_vT = np.ascontiguousarray(w_v.T)  # [F+3, F]
    w_vTm = np.ascontiguousarray(w_vT[:F])
    bvec = w_v[:, :F] @ bet  # [F]
    wv_tail = np.ascontiguousarray(
        np.concatenate([w_vT[F:], bvec[None, :]], axis=0)
    )
    me = np.exp(mixing)
    m = me / me.sum(axis=0, keepdims=True)
    m_n = np.ascontiguousarray(m.reshape(-1))  # [m00, m01, m10, m11]

    return [
        {
            "h": h,
            "x": x,
            "h_loc": np.ascontiguousarray(h[c * R : (c + 1) * R]),
            "x_loc": np.ascontiguousarray(x[c * R : (c + 1) * R]),
            "w_k8": w_k8,
            "w_q8": w_q8,
            "w_vTm": w_vTm,
            "wv_tail": wv_tail,
            "m_n": m_n,
            "ln_gamma": gam,
            "ln_beta": np.ascontiguousarray(bet),
        }
        for c in range(NCORES)
    ]


def kernel(h, x, w_k, w_q, w_v, mixing, ln_gamma, ln_beta):
    from concourse.bass_utils import run_bass_kernel_spmd

    if "nc" not in _CACHE:
        _CACHE["nc"] = _build()
    nc = _CACHE["nc"]

    in_maps = _make_in_maps(
        {
            "h": h,
            "x": x,
            "w_k": w_k,
            "w_q": w_q,
            "w_v": w_v,
            "mixing": mixing,
            "ln_gamma": ln_gamma,
            "ln_beta": ln_beta,
        }
    )
    res = run_bass_kernel_spmd(nc, in_maps, list(range(NCORES))).results
    h_out = np.concatenate([res[c]["h_out"] for c in range(NCORES)], axis=0)
    x_out = np.concatenate([res[c]["x_out"] for c in range(NCORES)], axis=0)
    return (h_out, x_out)


# revision 28
# speedup vs baseline: 1.4793x; 1.0443x over previous
# Trainium2 Bass kernel for the BronxLayer GNN message-passing problem (v2.1).
#
# Reference math (fp32):
#   hn = LayerNorm(h)*gamma + beta ; xn = x / max(|x|_1, 1e-12)
#   k = hn@w_k.T ; q = hn@w_q.T ; a_h = softmax(k@q.T/16) ; a_x = xn@xn.T
#   i = [diag(a_x), rowsum(a_x), rowstd(a_x, ddof=1)] ; m = softmax(mixing, 0)
#   x_out = (m00*a_x + m10*a_h)@xn + x
#   h_agg = m01*(a_x@hn) + m11*(a_h.T@hn)          (a_x symmetric)
#   h_out = elu([h_agg|i]@w_v.T) + h
#
# Sharding: nodes row-sharded over 8 cores.  The only cross-core term,
# m11*(a_h.T@hn), is reduced with one bf16 ReduceScatter of
# partial = E.T @ [hn*m11/rowsum | m11/rowsum], triggered right after the
# h-projection path so it overlaps the whole x/aggregation phase.
#
# Perf structure (v2.1):
#  - fp8e4+DoubleRow matmuls for qT/kT/S/a_xc/ps_xs/Gram/GX; E kept bf16 so
#    ET comes from 4 hardware DMA transposes instead of 64 PE transposes.
#  - per-chunk pipelined LayerNorm: bn_stats/bn_aggr + a bit-trick Newton
#    rsqrt on the vector engine (no scalar Sqrt -> scalar act table stays
#    on Exp the whole kernel, zero table reloads after the first).
#  - engine balance: applies on gpsimd, psum->sbuf casts split
#    scalar/vector, input DMA issue split across the two HWDGE queues.
import sys

if "/opt/trn_rl_repo" not in sys.path:
    sys.path.insert(0, "/opt/trn_rl_repo")

import numpy as np

N, F = 4096, 256
NCORES = 8
R = N // NCORES  # 512
P = 128
MT = R // P      # 4
NT = N // P      # 32
FT = F // P      # 2
NCH = N // 512   # 8
FP = 272         # partial width: F cols + colsum col + pad (16B-aligned)
LN_EPS = 1e-5
L1_EPS = 1e-12
WS = 8.0                     # w_k / w_q prescale (host)
SEXP = (1.0 / 16.0) / (WS * WS)
SA = 256.0                   # xn fp8 scale
SA2 = 8192.0                 # a_x fp8 scale (psum SA^2*a_x -> copy scale 1/8)
SG = 1024.0                  # Gram fp8 scale
SS = 0.25                    # colsum fp8 scale relative to SA*colsum
RSQRT_MAGIC = 0x5F3759DF

_CACHE = {}


def _build():
    import contextlib

    import concourse.mybir as mybir
    import concourse.tile as tile
    from concourse import bacc
    from concourse.bass import ds, ts
    from concourse.masks import make_identity

    f32 = mybir.dt.float32
    f32r = mybir.dt.float32r
    bf16 = mybir.dt.bfloat16
    f8 = mybir.dt.float8e4
    i32 = mybir.dt.int32
    AF = mybir.ActivationFunctionType
    OP = mybir.AluOpType
    AX = mybir.AxisListType
    DR = mybir.MatmulPerfMode.DoubleRow

    nc = bacc.Bacc(None, num_devices=NCORES)

    h_ext = nc.declare_dram_parameter("h", [N, F], f32, isOutput=False)
    x_ext = nc.declare_dram_parameter("x", [N, F], f32, isOutput=False)
    hloc_ext = nc.declare_dram_parameter("h_loc", [R, F], f32, isOutput=False)
    xloc_ext = nc.declare_dram_parameter("x_loc", [R, F], f32, isOutput=False)
    wk_ext = nc.declare_dram_parameter("w_k8", [F, F], f32, isOutput=False)
    wq_ext = nc.declare_dram_parameter("w_q8", [F, F], f32, isOutput=False)
    wv_ext = nc.declare_dram_parameter("w_vTm", [F, F], f32r, isOutput=False)
    wvt_ext = nc.declare_dram_parameter("wv_tail", [4, F], f32r, isOutput=False)
    mn_ext = nc.declare_dram_parameter("m_n", [4], f32, isOutput=False)
    gam_ext = nc.declare_dram_parameter("ln_gamma", [F], f32, isOutput=False)
    bet_ext = nc.declare_dram_parameter("ln_beta", [F], f32, isOutput=False)
    hout_ext = nc.declare_dram_parameter("h_out", [R, F], f32, isOutput=True)
    xout_ext = nc.declare_dram_parameter("x_out", [R, F], f32, isOutput=True)

    h_v = h_ext.rearrange("(o p) f -> p o f", p=P)
    x_v = x_ext.rearrange("(o p) f -> p o f", p=P)
    hloc_v = hloc_ext.rearrange("(o p) f -> p o f", p=P)
    xloc_v = xloc_ext.rearrange("(o p) f -> p o f", p=P)
    hout_v = hout_ext.rearrange("(o p) f -> p o f", p=P)
    xout_v = xout_ext.rearrange("(o p) f -> p o f", p=P)

    NS = NT + MT

    with tile.TileContext(nc) as tc, contextlib.ExitStack() as ctx:
        const = ctx.enter_context(tc.tile_pool(name="const", bufs=1))
        persist = ctx.enter_context(tc.tile_pool(name="persist", bufs=1))
        dram = ctx.enter_context(tc.tile_pool(name="dram", bufs=1, space="DRAM"))
        small = ctx.enter_context(tc.tile_pool(name="small", bufs=3))

        # ---------------- persistent tensors ----------------
        h_in = persist.tile([P, NT, F], f32, name="h_in")
        x_in = persist.tile([P, NT, F], f32, name="x_in")
        h_lin = persist.tile([P, MT, F], f32, name="h_lin")
        x_lin = persist.tile([P, MT, F], f32, name="x_lin")
        xh_b = persist.tile([P, NT, 2 * F], f8, name="xh_b")
        xnT = persist.tile([P, FT, N], f8, name="xnT")
        kT_loc = persist.tile([P, FT, R], f8, name="kT_loc")
        xnT_loc = persist.tile([P, FT, R], f8, name="xnT_loc")
        E = persist.tile([P, MT, N], bf16, name="E")
        ET_all = persist.tile([P, NT, R], bf16, name="ET_all")
        hn_loc = persist.tile([P, MT, F], bf16, name="hn_loc")
        xn_loc_b = persist.tile([P, MT, F], bf16, name="xn_loc_b")
        hn_scaled = persist.tile([P, MT, FP], bf16, name="hn_scaled")
        rowsum_parts = persist.tile([P, MT, NCH], f32, name="rowsum_parts")
        recip_r = persist.tile([P, MT], f32, name="recip_r")
        diag = persist.tile([P, MT], f32, name="diag")
        srow = persist.tile([P, MT], f32, name="srow")
        stdv = persist.tile([P, MT], f32, name="stdv")
        G8 = persist.tile([P, FT, F], f8, name="G8")
        GXT = persist.tile([P, MT, F], bf16, name="GXT")
        i_cols = persist.tile([P, MT, 4], f32, name="i_cols")
        i_T = persist.tile([P, R], f32r, name="i_T")
        rs_sb = persist.tile([P, MT, FP], bf16, name="rs_sb")
        h_aggT = persist.tile([P, FT, R], f32r, name="h_aggT")
        l1_t = persist.tile([P, NS], f32, name="l1_t")
        rl1s_t = persist.tile([P, NS], f32, name="rl1s_t")

        partial_dram = dram.tile([N, FP], bf16, name="partial_dram")
        rs_dram = dram.tile([R, FP], bf16, name="rs_dram")
        pd_v = partial_dram.rearrange("(a p) f -> p a f", p=P)

        # ---------------- DMA issue: h on sync, x/weights on scalar --------
        nc.sync.dma_start(h_lin[:], hloc_v[:])
        for c in range(NCH):
            nc.sync.dma_start(h_in[:, ds(4 * c, 4), :], h_v[:, ds(4 * c, 4)])

        m_bc = const.tile([P, 4], f32, name="m_bc")
        nc.sync.dma_start(m_bc[:], mn_ext.rearrange("a -> () a").to_broadcast((P, 4)))
        M00, M01, M10, M11 = (m_bc[:, j : j + 1] for j in range(4))
        gam_f = const.tile([P, FT, 1], f32, name="gam_f")
        nc.sync.dma_start(gam_f[:, :, 0], gam_ext.rearrange("(o p) -> p o", p=P))
        bet_f = const.tile([P, FT, 1], f32, name="bet_f")
        nc.sync.dma_start(bet_f[:, :, 0], bet_ext.rearrange("(o p) -> p o", p=P))
        wvT = const.tile([P, FT, F], f32r, name="wvT")
        nc.scalar.dma_start(wvT[:], wv_ext.rearrange("(o p) f -> p o f", p=P))
        wvT3 = const.tile([P, F], f32r, name="wvT3")
        nc.vector.memset(wvT3[:].bitcast(f32), 0.0)
        nc.scalar.dma_start(wvT3[:4, :], wvt_ext[:])

        ident8 = const.tile([P, P], f8, name="ident8")
        make_identity(nc, ident8)
        ident_b = const.tile([P, P], bf16, name="ident_b")
        make_identity(nc, ident_b)
        ident_f = const.tile([P, P], f32, name="ident_f")
        make_identity(nc, ident_f)

        wq8 = const.tile([P, FT, F], f8, name="wq8")
        wk8 = const.tile([P, FT, F], f8, name="wk8")

        rm01 = const.tile([P, 1], f32, name="rm01")
        nc.vector.reciprocal(rm01[:], M01)
        gam_eff = const.tile([P, FT, 1], f32, name="gam_eff")
        for ft in range(FT):
            nc.vector.tensor_tensor(gam_eff[:, ft], gam_f[:, ft], rm01[:], OP.mult)
        m00s = const.tile([P, 1], f32, name="m00s")
        nc.vector.tensor_scalar_mul(m00s[:], M00, 1.0 / (SA2 * SA))
        m10s = const.tile([P, 1], f32, name="m10s")
        nc.vector.tensor_scalar_mul(m10s[:], M10, 1.0 / SA)

        def newton_rsqrt(ve, nrows, tag):
            """1/sqrt(ve) on the vector engine (ve: [P, nrows] f32 AP,
            destroyed).  Returns an f32 AP view.  ~7 small DVE ops."""
            ish = small.tile([P, nrows], i32, name="nr_i" + tag, tag="nri" + tag)
            nc.vector.tensor_scalar(
                ish[:], ve.bitcast(i32), scalar1=1, scalar2=None,
                op0=OP.logical_shift_right,
            )
            y0 = small.tile([P, nrows], i32, name="nr_y" + tag, tag="nry" + tag)
            nc.vector.tensor_scalar(
                y0[:], ish[:], scalar1=-1, scalar2=RSQRT_MAGIC, op0=OP.mult,
                op1=OP.add,
            )
            yf = y0[:].bitcast(f32)
            hv = small.tile([P, nrows], f32, name="nr_h" + tag, tag="nrh" + tag)
            nc.vector.tensor_scalar_mul(hv[:], ve, 0.5)
            t = small.tile([P, nrows], f32, name="nr_t" + tag, tag="nrt" + tag)
            for _ in range(2):
                nc.vector.tensor_tensor(t[:], yf, yf, OP.mult)
                nc.vector.tensor_tensor(t[:], t[:], hv[:], OP.mult)
                nc.vector.tensor_scalar(
                    t[:], t[:], scalar1=-1.0, scalar2=1.5, op0=OP.mult, op1=OP.add
                )
                nc.vector.tensor_tensor(yf, yf, t[:], OP.mult)
            return yf

        def ln_coeffs(src_blk, nrows, tag):
            """bn_stats -> (rstm, nmrm) = (m01/sd, -mean*m01/sd) and
            (rstd, nmr) plain views for the local tile."""
            st6 = small.tile([P, nrows, 6], f32, name="st6" + tag, tag="st6")
            for j in range(nrows):
                nc.vector.bn_stats(st6[:, j], src_blk[:, j])
            mv = small.tile([P, nrows, 2], f32, name="mv" + tag, tag="mv")
            for j in range(nrows):
                nc.vector.bn_aggr(mv[:, j], st6[:, j])
            ve = small.tile([P, nrows], f32, name="ve" + tag, tag="ve")
            nc.vector.tensor_scalar_add(ve[:], mv[:, :, 1], LN_EPS)
            rstd = newton_rsqrt(ve[:], nrows, tag)
            rstm = small.tile([P, nrows], f32, name="rsm" + tag, tag="rsm")
            nc.vector.tensor_tensor(
                rstm[:], rstd, M01.to_broadcast((P, nrows)), OP.mult
            )
            nmrm = small.tile([P, nrows], f32, name="nmm" + tag, tag="nmm")
            nc.vector.scalar_tensor_tensor(
                nmrm[:], mv[:, :, 0], -1.0, rstm[:], OP.mult, OP.mult
            )
            return rstd, mv, rstm, nmrm

        # ============ h phase ============
        with tc.tile_pool(name="p1", bufs=1, space="PSUM") as p1, \
             tc.tile_pool(name="sc1", bufs=1) as sc1:
            # weight staging + fp8 conversion (gpsimd; vector stays free)
            wq_st = sc1.tile([P, FT, F], f32, name="wq_st", tag="wst", bufs=1)
            nc.sync.dma_start(wq_st[:], wq_ext.rearrange("(o p) f -> p o f", p=P))
            nc.vector.tensor_copy(out=wq8[:], in_=wq_st[:])
            wk_st = sc1.tile([P, FT, F], f32, name="wk_st", tag="wst", bufs=1)
            nc.sync.dma_start(wk_st[:], wk_ext.rearrange("(o p) f -> p o f", p=P))
            nc.vector.tensor_copy(out=wk8[:], in_=wk_st[:])
            # ---- local tile: hn_loc, lh8, hnT_l, kT_loc ----
            rstd_l, mv_l, rstm_l, nmrm_l = ln_coeffs(h_lin[:], MT, "l")
            lh8 = sc1.tile([P, MT, F], f8, name="lh8")
            for j in range(MT):
                nc.vector.tensor_scalar(
                    lh8[:, j], h_lin[:, j],
                    scalar1=rstm_l[:, j : j + 1], scalar2=nmrm_l[:, j : j + 1],
                    op0=OP.mult, op1=OP.add,
                )
            nmr_l = small.tile([P, MT], f32, name="nmr_l", tag="nmrl", bufs=1)
            nc.vector.scalar_tensor_tensor(
                nmr_l[:], mv_l[:, :, 0], -1.0, rstd_l, OP.mult, OP.mult
            )
            for j in range(MT):
                nc.vector.tensor_scalar(
                    hn_loc[:, j], h_lin[:, j],
                    scalar1=rstd_l[:, j : j + 1], scalar2=nmr_l[:, j : j + 1],
                    op0=OP.mult, op1=OP.add,
                )
            hnT_l = sc1.tile([P, FT, R], f8, name="hnT_l")
            for ft in range(FT):
                ps_t = p1.tile([P, R, 2], f8, name="ps_tl", tag="tp", bufs=2)
                for j in range(MT):
                    nc.tensor.transpose(
                        ps_t[:, ts(j, P), 0], lh8[:, j, ds(128 * ft, P)], ident8[:]
                    )
                nc.scalar.activation(
                    hnT_l[:, ft], ps_t[:, :, 0], AF.Identity,
                    bias=bet_f[:, ft], scale=gam_eff[:, ft],
                )
            for fo in range(FT):
                ps_k = p1.tile([P, R], f32, name="ps_k", tag="mm", bufs=4)
                nc.tensor.matmul(
                    ps_k[:], wk8[:, 0:2, ds(128 * fo, P)], hnT_l[:, 0:2, :],
                    start=True, stop=True, perf_mode=DR,
                )
                nc.scalar.activation(kT_loc[:, fo], ps_k[:], AF.Copy)

            # ---- global chunks, fully pipelined ----
            for c in range(NCH):
                blk = h_in[:, ds(4 * c, 4), :]
                _, _, rstm, nmrm = ln_coeffs(blk, 4, "c")
                for j in range(4):
                    nt = 4 * c + j
                    nc.vector.tensor_scalar(
                        xh_b[:, nt, F : 2 * F], h_in[:, nt],
                        scalar1=rstm[:, j : j + 1], scalar2=nmrm[:, j : j + 1],
                        op0=OP.mult, op1=OP.add,
                    )
                hnT_c = sc1.tile([P, FT, R], f8, name="hnT_c", tag="hnT", bufs=2)
                for ft in range(FT):
                    ps_t = p1.tile([P, R, 2], f8, name="ps_t", tag="tp", bufs=2)
                    for j in range(4):
                        nt = 4 * c + j
                        nc.tensor.transpose(
                            ps_t[:, ts(j, P), 0],
                            xh_b[:, nt, ds(F + 128 * ft, P)],
                            ident8[:],
                        )
                    nc.vector.tensor_scalar(
                        hnT_c[:, ft], ps_t[:, :, 0],
                        scalar1=gam_eff[:, ft], scalar2=bet_f[:, ft],
                        op0=OP.mult, op1=OP.add,
                    )
                qT_c = sc1.tile([P, FT, R], f8, name="qT_c", tag="qTc", bufs=2)
                for fo in range(FT):
                    ps_q = p1.tile([P, R], f32, name="ps_q", tag="mm", bufs=4)
                    nc.tensor.matmul(
                        ps_q[:], wq8[:, 0:2, ds(128 * fo, P)], hnT_c[:, 0:2, :],
                        start=True, stop=True, perf_mode=DR,
                    )
                    nc.scalar.activation(qT_c[:, fo], ps_q[:], AF.Copy)
                for mt in range(MT):
                    ps_s = p1.tile([P, R], f32, name="ps_s", tag="mm", bufs=4)
                    nc.tensor.matmul(
                        ps_s[:],
                        kT_loc[:, 0:2, ds(128 * mt, P)],
                        qT_c[:, 0:2, :],
                        start=True, stop=True, perf_mode=DR,
                    )
                    nc.scalar.activation(
                        E[:, mt, ds(512 * c, 512)], ps_s[:], AF.Exp,
                        scale=SEXP, accum_out=rowsum_parts[:, mt, c : c + 1],
                    )

            # x inputs stream in after h owns the early HBM bandwidth
            with tc.tile_wait_until(0.020):
                nc.scalar.dma_start(x_lin[:], xloc_v[:])
                for c in range(NCH):
                    nc.scalar.dma_start(
                        x_in[:, ds(4 * c, 4), :], x_v[:, ds(4 * c, 4)]
                    )

            # ---- rowsums -> hn_scaled ----
            rs1 = small.tile([P, MT], f32, name="rs1", tag="rs1")
            nc.vector.tensor_reduce(rs1[:], rowsum_parts[:], AX.X, OP.add)
            nc.vector.reciprocal(recip_r[:], rs1[:])
            sch = small.tile([P, MT], f32, name="sch", tag="sch", bufs=1)
            nc.vector.tensor_tensor(
                sch[:], recip_r[:], M11.to_broadcast((P, MT)), OP.mult
            )
            nc.vector.memset(hn_scaled[:].bitcast(f32), 0.0)
            for mt in range(MT):
                nc.vector.tensor_scalar_mul(
                    hn_scaled[:, mt, 0:F], hn_loc[:, mt], sch[:, mt : mt + 1]
                )
                nc.vector.tensor_copy(
                    out=hn_scaled[:, mt, F : F + 1], in_=sch[:, mt : mt + 1]
                )

            # ---- partial = E.T @ hn_scaled -> bf16 -> DRAM (scalar stg) ----
            for g in range(8):
                stg = sc1.tile([P, 4, FP], bf16, name="stg", tag="stg", bufs=2)
                for k in range(4):
                    ic = 4 * g + k
                    ps_p = p1.tile([P, FP], f32, name="ps_p", tag="mm", bufs=4)
                    for jt in range(MT):
                        nc.tensor.matmul(
                            ps_p[:],
                            E[:, jt, ds(128 * ic, P)],
                            hn_scaled[:, jt, :],
                            start=(jt == 0), stop=(jt == MT - 1),
                        )
                    nc.scalar.activation(stg[:, k], ps_p[:], AF.Copy)
                nc.sync.dma_start(pd_v[:, ds(4 * g, 4), :], stg[:])

        # ET via 4 hardware DMA transposes (bf16), overlapping the collective
        for mt in range(MT):
            nc.sync.dma_start_transpose(ET_all[:, :, ds(128 * mt, P)], E[:, mt, :])

        nc.gpsimd.collective_compute(
            "ReduceScatter",
            mybir.AluOpType.add,
            replica_groups=[list(range(NCORES))],
            ins=[partial_dram[:]],
            outs=[rs_dram[:]],
        )
        nc.sync.dma_start(rs_sb[:], rs_dram.rearrange("(o p) f -> p o f", p=P))

        # ============ x phase ============
        with tc.tile_pool(name="p2", bufs=1, space="PSUM") as p2, \
             tc.tile_pool(name="sc2", bufs=1) as sc2:
            # L1 (vector has slack while partial streams out)
            with tc.tile_wait_until(0.045):
                for c in range(NCH):
                    nc.vector.tensor_reduce(
                        l1_t[:, ds(4 * c, 4)], x_in[:, ds(4 * c, 4), :],
                        AX.X, OP.add, apply_absolute_value=True,
                    )
                nc.vector.tensor_reduce(
                    l1_t[:, ds(NT, MT)], x_lin[:], AX.X, OP.add,
                    apply_absolute_value=True,
                )
            nc.vector.tensor_scalar_max(l1_t[:], l1_t[:], L1_EPS)
            nc.vector.reciprocal(rl1s_t[:], l1_t[:])
            nc.vector.tensor_scalar_mul(rl1s_t[:], rl1s_t[:], SA)

            # local rows: xn_loc_b (bf16), x8_l (fp8), diag, xnT_loc
            rl1p = small.tile([P, MT], f32, name="rl1p", tag="rl1p", bufs=1)
            nc.vector.tensor_scalar_mul(rl1p[:], rl1s_t[:, ds(NT, MT)], 1.0 / SA)
            sqj = sc2.tile([P, MT, F], f8, name="sqj")
            ssq = small.tile([P, MT], f32, name="ssq", tag="ssq", bufs=1)
            for j in range(MT):
                nc.vector.tensor_scalar_mul(
                    xn_loc_b[:, j], x_lin[:, j], rl1p[:, j : j + 1]
                )
                nc.scalar.activation(
                    sqj[:, j], x_lin[:, j], AF.Square,
                    accum_out=ssq[:, j : j + 1],
                )
            t0 = small.tile([P, MT], f32, name="t0d", tag="t0d", bufs=1)
            nc.vector.tensor_tensor(t0[:], rl1p[:], rl1p[:], OP.mult)
            nc.vector.tensor_tensor(diag[:], ssq[:], t0[:], OP.mult)
            x8_l = sc2.tile([P, MT, F], f8, name="x8_l")
            for j in range(MT):
                nc.vector.tensor_scalar_mul(
                    x8_l[:, j], x_lin[:, j], rl1s_t[:, NT + j : NT + j + 1]
                )
            for ft in range(FT):
                ps_t = p2.tile([P, R, 2], f8, name="ps_xtl", tag="tp", bufs=2)
                for j in range(MT):
                    nc.tensor.transpose(
                        ps_t[:, ts(j, P), 0], x8_l[:, j, ds(128 * ft, P)], ident8[:]
                    )
                nc.vector.tensor_copy(out=xnT_loc[:, ft], in_=ps_t[:, :, 0])

            # global: xh_b x-half (gpsimd), xnT (vector casts), Gram (DR)
            ps_G = p2.tile([P, 2 * F], f32, name="ps_G", tag="G", bufs=1)
            for c in range(NCH):
                for j in range(4):
                    nt = 4 * c + j
                    nc.vector.tensor_scalar_mul(
                        xh_b[:, nt, 0:F], x_in[:, nt], rl1s_t[:, nt : nt + 1]
                    )
                for ft in range(FT):
                    ps_t = p2.tile([P, R, 2], f8, name="ps_xt", tag="tp", bufs=2)
                    for j in range(4):
                        nt = 4 * c + j
                        nc.tensor.transpose(
                            ps_t[:, ts(j, P), 0],
                            xh_b[:, nt, ds(128 * ft, P)],
                            ident8[:],
                        )
                    nc.vector.tensor_copy(
                        out=xnT[:, ft, ds(512 * c, 512)], in_=ps_t[:, :, 0]
                    )
                for pr in range(2):
                    nt0 = 4 * c + 2 * pr
                    for m in range(FT):
                        nc.tensor.matmul(
                            ps_G[:, ts(m, F)],
                            xh_b[:, nt0 : nt0 + 2, ds(128 * m, P)],
                            xh_b[:, nt0 : nt0 + 2, 0:F],
                            start=(c == 0 and pr == 0),
                            stop=(c == NCH - 1 and pr == 1),
                            perf_mode=DR,
                        )
            for m in range(FT):
                nc.vector.tensor_scalar_mul(
                    G8[:, m], ps_G[:, ts(m, F)], SG / (SA * SA)
                )

        # ============ aggregation ============
        with tc.tile_pool(name="p3", bufs=1, space="PSUM") as p3, \
             tc.tile_pool(name="sc3", bufs=1) as sc3:
            ps_xs = [
                p3.tile([P, R], f32, name=f"ps_xs{mt}", tag=f"xs{mt}", bufs=1)
                for mt in range(MT)
            ]
            with tc.tile_pool(name="p3a", bufs=1, space="PSUM") as p3a:
                for icp in range(16):
                    axc2 = sc3.tile([P, 2, R], f8, name="axc2", tag="axc", bufs=3)
                    for q in range(2):
                        ic = 2 * icp + q
                        ps_ax = p3a.tile([P, R], f32, name="ps_ax", tag="ax",
                                         bufs=4)
                        nc.tensor.matmul(
                            ps_ax[:], xnT[:, 0:2, ds(128 * ic, P)],
                            xnT_loc[:, 0:2, :],
                            start=True, stop=True, perf_mode=DR,
                        )
                        eng = nc.scalar if q == 0 else nc.vector
                        if q == 0:
                            nc.scalar.activation(
                                axc2[:, q], ps_ax[:], AF.Copy,
                                scale=SA2 / (SA * SA),
                            )
                        else:
                            nc.vector.tensor_scalar_mul(
                                axc2[:, q], ps_ax[:], SA2 / (SA * SA)
                            )
                    for mt in range(MT):
                        nc.tensor.matmul(
                            ps_xs[mt][:],
                            axc2[:, 0:2, ds(128 * mt, P)],
                            xh_b[:, ds(2 * icp, 2), :],
                            start=(icp == 0), stop=(icp == 15), perf_mode=DR,
                        )

            # bslcT: (E @ xn).T accumulated f-major (fp8 lhsT x bf16 rhs)
            p3b_cm = tc.tile_pool(name="p3b", bufs=1, space="PSUM")
            p3b = p3b_cm.__enter__()
            ps_bT = [
                p3b.tile([P, R], f32, name=f"ps_bT{fs}", tag=f"bT{fs}", bufs=1)
                for fs in range(FT)
            ]
            for ic in range(NT):
                for fs in range(FT):
                    nc.tensor.matmul(
                        ps_bT[fs][:],
                        xh_b[:, ic, ds(128 * fs, P)],
                        ET_all[:, ic, :],
                        start=(ic == 0), stop=(ic == NT - 1),
                    )
            bT_sb = sc3.tile([P, FT, R], bf16, name="bT_sb")
            for fs in range(FT):
                nc.vector.tensor_copy(out=bT_sb[:, fs], in_=ps_bT[fs][:])

            # ---- x_out combine ----
            for mt in range(MT):
                ps_bb = p3b.tile([P, F], bf16, name="ps_bb", tag="w", bufs=2)
                for fs in range(FT):
                    nc.tensor.transpose(
                        ps_bb[:, ts(fs, P)], bT_sb[:, fs, ds(128 * mt, P)],
                        ident_b[:],
                    )
                xo = small.tile([P, F], f32, name="xo", tag="xo", bufs=2)
                nc.vector.tensor_scalar_mul(xo[:], ps_xs[mt][:, 0:F], m00s[:])
                scb = small.tile([P, 1], f32, name="scb", tag="scb")
                nc.vector.tensor_tensor(
                    scb[:], recip_r[:, mt : mt + 1], m10s[:], OP.mult
                )
                tb = small.tile([P, F], f32, name="tb", tag="wkb", bufs=2)
                nc.vector.tensor_scalar_mul(tb[:], ps_bb[:], scb[:])
                nc.vector.tensor_tensor(xo[:], xo[:], tb[:], OP.add)
                nc.vector.tensor_tensor(xo[:], xo[:], x_lin[:, mt], OP.add)
                nc.sync.dma_start(xout_v[:, mt], xo[:])

            # ---- late stats: srow / GX -> sumsq -> std ----
            s_f = small.tile([P, FT], f32, name="s_f", tag="s_f", bufs=1)
            nc.vector.tensor_reduce(s_f[:], xnT[:], AX.X, OP.add)
            s8 = small.tile([P, FT], f8, name="s8", tag="s8", bufs=1)
            nc.vector.tensor_scalar_mul(s8[:], s_f[:], SS)
            ps_sr = p3b.tile([P, MT], f32, name="ps_sr", tag="w", bufs=2)
            for mt in range(MT):
                for ft in range(FT):
                    nc.tensor.matmul(
                        ps_sr[:, mt : mt + 1],
                        xnT_loc[:, ft, ds(128 * mt, P)],
                        s8[:, ft : ft + 1],
                        start=(ft == 0), stop=(ft == 1),
                    )
            nc.vector.tensor_scalar_mul(srow[:], ps_sr[:], 1.0 / (SA * SA * SS))
            GX_sb = sc3.tile([P, FT, R], bf16, name="GX_sb")
            for fs in range(FT):
                ps_gx = p3b.tile([P, R], f32, name="ps_gx", tag="w", bufs=2)
                nc.tensor.matmul(
                    ps_gx[:], G8[:, 0:2, ds(128 * fs, P)], xnT_loc[:, 0:2, :],
                    start=True, stop=True, perf_mode=DR,
                )
                nc.vector.tensor_copy(out=GX_sb[:, fs], in_=ps_gx[:])
            for mt in range(MT):
                ps_gxt = p3b.tile([P, F], bf16, name="ps_gxt", tag="w", bufs=2)
                for fs in range(FT):
                    nc.tensor.transpose(
                        ps_gxt[:, ts(fs, P)], GX_sb[:, fs, ds(128 * mt, P)],
                        ident_b[:],
                    )
                nc.vector.tensor_copy(out=GXT[:, mt], in_=ps_gxt[:])
            sumsq = small.tile([P, MT], f32, name="sumsq", tag="sumsq", bufs=1)
            for mt in range(MT):
                tmp2 = small.tile([P, F], f32, name="tmp2", tag="wka", bufs=2)
                nc.vector.tensor_tensor(
                    tmp2[:], xn_loc_b[:, mt], GXT[:, mt], OP.mult
                )
                nc.vector.tensor_reduce(
                    sumsq[:, mt : mt + 1], tmp2[:], AX.X, OP.add
                )
            sr2 = small.tile([P, MT], f32, name="sr2", tag="sr2", bufs=1)
            nc.vector.scalar_tensor_tensor(
                sr2[:], srow[:], 1.0 / (N * (N - 1.0)), srow[:], OP.mult, OP.mult
            )
            t1 = small.tile([P, MT], f32, name="t1", tag="t1", bufs=1)
            nc.vector.scalar_tensor_tensor(
                t1[:], sumsq[:], 1.0 / (SG * SA * (N - 1.0)), sr2[:],
                OP.mult, OP.subtract,
            )
            # std = t1 * rsqrt(t1) (vector; avoids a scalar Sqrt table load)
            t1c = small.tile([P, MT], f32, name="t1c", tag="t1c", bufs=1)
            nc.vector.tensor_scalar_max(t1c[:], t1[:], 1e-30)
            rsq = newton_rsqrt(t1c[:], MT, "s")
            nc.vector.tensor_tensor(stdv[:], t1[:], rsq, OP.mult)

            # ---- h path ----
            h_agg_s = sc3.tile([P, MT, F], bf16, name="h_agg_s")
            for mt in range(MT):
                nc.vector.scalar_tensor_tensor(
                    h_agg_s[:, mt], ps_xs[mt][:, F : 2 * F], 1.0 / SA2,
                    rs_sb[:, mt, 0:F], OP.mult, OP.add,
                )
                nc.vector.tensor_copy(out=i_cols[:, mt, 0:1], in_=diag[:, mt : mt + 1])
                nc.vector.tensor_copy(out=i_cols[:, mt, 1:2], in_=srow[:, mt : mt + 1])
                nc.vector.tensor_copy(out=i_cols[:, mt, 2:3], in_=stdv[:, mt : mt + 1])
                c4 = small.tile([P, 1], f32, name="c4", tag="c4", bufs=4)
                nc.vector.tensor_tensor(c4[:], srow[:, mt : mt + 1], M01, OP.mult)
                nc.vector.tensor_tensor(
                    i_cols[:, mt, 3:4], rs_sb[:, mt, F : F + 1], c4[:], OP.add
                )
            nc.vector.memset(i_T[:].bitcast(f32), 0.0)
            for mt in range(MT):
                ps_i = p3b.tile([4, P], f32, name="ps_i", tag="w", bufs=2)
                nc.tensor.transpose(ps_i[:], i_cols[:, mt], ident_f[:])
                nc.vector.tensor_copy(out=i_T[:4, ds(128 * mt, P)], in_=ps_i[:])
            for ft in range(FT):
                ps_hat = p3b.tile([P, R], bf16, name="ps_hat", tag="w", bufs=2)
                for mt in range(MT):
                    nc.tensor.transpose(
                        ps_hat[:, ts(mt, P)], h_agg_s[:, mt, ds(128 * ft, P)],
                        ident_b[:],
                    )
                nc.scalar.activation(
                    h_aggT[:, ft], ps_hat[:], AF.Copy, scale=gam_f[:, ft]
                )
            for mt in range(MT):
                ps_h = p3b.tile([P, F], f32, name="ps_h", tag="w", bufs=2)
                for k in range(FT):
                    nc.tensor.matmul(
                        ps_h[:], h_aggT[:, k, ds(128 * mt, P)], wvT[:, k],
                        start=(k == 0), stop=False,
                    )
                nc.tensor.matmul(
                    ps_h[:], i_T[:, ds(128 * mt, P)], wvT3[:],
                    start=False, stop=True,
                )
                vmin = small.tile([P, F], f32, name="vmin", tag="wka", bufs=2)
                nc.vector.tensor_scalar_min(vmin[:], ps_h[:], 0.0)
                ev = small.tile([P, F], f32, name="ev", tag="wkb", bufs=2)
                nc.scalar.activation(ev[:], vmin[:], AF.Exp)
                vmax = small.tile([P, F], f32, name="vmax", tag="wka", bufs=2)
                nc.vector.tensor_scalar_max(vmax[:], ps_h[:], 0.0)
                ho = small.tile([P, F], f32, name="ho", tag="ho", bufs=2)
                nc.vector.tensor_tensor(ho[:], ev[:], vmax[:], OP.add)
                nc.vector.tensor_scalar_add(ho[:], ho[:], -1.0)
                nc.vector.tensor_tensor(ho[:], ho[:], h_lin[:, mt], OP.add)
                nc.sync.dma_start(hout_v[:, mt], ho[:])
            p3b_cm.__exit__(None, None, None)

    nc.finalize()
    return nc


def _make_in_maps(inputs):
    h = np.ascontiguousarray(inputs["h"], dtype=np.float32)
    x = np.ascontiguousarray(inputs["x"], dtype=np.float32)
    w_k = np.asarray(inputs["w_k"], np.float32)
    w_q = np.asarray(inputs["w_q"], np.float32)
    w_v = np.asarray(inputs["w_v"], np.float32)
    mixing = np.asarray(inputs["mixing"], np.float32)
    gam = np.ascontiguousarray(inputs["ln_gamma"], dtype=np.float32)
    bet = np.asarray(inputs["ln_beta"], np.float32)

    w_k8 = np.ascontiguousarray(w_k.T * WS)
    w_q8 = np.ascontiguousarray(w_q.T * WS)
    w_vT = np.ascontiguousarray(w_v.T)  # [F+3, F]
    w_vTm = np.ascontiguousarray(w_vT[:F])
    bvec = w_v[:, :F] @ bet  # [F]
    wv_tail = np.ascontiguousarray(
        np.concatenate([w_vT[F:], bvec[None, :]], axis=0)
    )
    me = np.exp(mixing)
    m = me / me.sum(axis=0, keepdims=True)
    m_n = np.ascontiguousarray(m.reshape(-1))  # [m00, m01, m10, m11]

    return [
        {
            "h": h,
            "x": x,
            "h_loc": np.ascontiguousarray(h[c * R : (c + 1) * R]),
            "x_loc": np.ascontiguousarray(x[c * R : (c + 1) * R]),
            "w_k8": w_k8,
            "w_q8": w_q8,
            "w_vTm": w_vTm,
            "wv_tail": wv_tail,
            "m_n": m_n,
            "ln_gamma": gam,
            "ln_beta": np.ascontiguousarray(bet),
        }
        for c in range(NCORES)
    ]


def kernel(h, x, w_k, w_q, w_v, mixing, ln_gamma, ln_beta):
    from concourse.bass_utils import run_bass_kernel_spmd

    if "nc" not in _CACHE:
        _CACHE["nc"] = _build()
    nc = _CACHE["nc"]

    in_maps = _make_in_maps(
        {
            "h": h,
            "x": x,
            "w_k": w_k,
            "w_q": w_q,
            "w_v": w_v,
            "mixing": mixing,
            "ln_gamma": ln_gamma,
            "ln_beta": ln_beta,
        }
    )
    res = run_bass_kernel_spmd(nc, in_maps, list(range(NCORES))).results
    h_out = np.concatenate([res[c]["h_out"] for c in range(NCORES)], axis=0)
    x_out = np.concatenate([res[c]["x_out"] for c in range(NCORES)], axis=0)
    return (h_out, x_out)
